# revision 27
# baseline (speedup 1.0000x reference)
r"""Circulant layer kernel for Trainium2 (8 NeuronCores).

Math: reference computes mv1 + mv2 where
  mv1 = batch_circulant(b) @ d,  mv2 = batch_circulant(d) @ b,
with d = des @ K, b = body @ K.  Both are the circular convolution of d and b
(circular convolution is commutative), so  out = 2 * circconv(d, b).

circconv via DFT:  out = 2 * Re(IDFT(DFT(d) * DFT(b))).  d and b are REAL,
so the spectrum is conjugate-symmetric and only frequencies 0..512 are
needed; paired frequencies 1..511 carry weight 4/N in the inverse, the
self-conjugate f=512 carries 2/N, and the f=0 (DC) term is a rank-1
correction added on the host during the unshard sum.

Sharding: core c owns the 64 frequencies f in [64c+1, 64c+64] (core 7's
last is f=512, whose sin column is identically zero).  Per core:
  KC_c   = K @ CC_c            (1024k x 128s)   fused projection+fwd DFT
  DT_c   = KC_c^T @ des^T      (128s x 128b)    \  shares stationary weights
  BT_c   = KC_c^T @ body^T     (128s x 128b)    /
  PT_c   = complex-mult(DT_c, BT_c)             (64f x 2 x 128b)
  part_c = (PT_c^T @ G_c)                       (128b x 1024)  inverse DFT
Host sums the 8 partials and adds the DC term (unshard).

Key structural facts this implementation is built around:
- walrus allows ONE sync wait per instruction, so every consumer's
  dependencies must collapse onto a single producer engine.  DMA-fed
  operands consumed together with engine-produced data (dbt, g) are
  staged through Vector copies.
- a PSUM accumulation chain's first matmul clears has_written bits for
  its WHOLE 2KB bank, and the Tile scheduler reorders matmuls, so bank
  sharing is only safe inside ONE chain with explicit order edges.
  Stage 1 packs four kb regions per bank inside one ordered chain;
  stages 2/4 use fresh banks so no bank is ever re-read after a rewrite.
- stage 1 runs j-outer with all 8 accumulators live, so each arriving
  K chunk (4 chunked DMAs) is consumed immediately.
- PE warmup (HAM clock ramp) is folded into stage 1: zero matmuls
  accumulate into the kb0 region before the real contributions.
"""

import numpy as np

import concourse.bass as bass
import concourse.mybir as mybir
import concourse.tile as tile
from concourse.bass_utils import run_bass_kernel_spmd
from concourse.tile_rust import add_dep_helper

B = 128        # batch
D_IN = 1024    # input feature dim (contraction k)
N = 1024       # output feature dim (conv length j)
N_CORES = 8
SH = 64             # frequencies per core (complex, from the half spectrum)
S = 2 * SH          # freq slots per core: [0:SH]=real(cos), [SH:2SH]=imag(-sin)
JC = N // 128       # 8 j-chunks (contraction of stage 1)
KB = D_IN // 128    # 8 k-blocks (output partitions of stage 1)
NQ = 4              # ktcc chunk DMAs (2 j-chunks each)

F32 = mybir.dt.float32
F32R = mybir.dt.float32r
BF16 = mybir.dt.bfloat16

# Matmul operand precision: "bf16" (fastest; ~5e-3 rel err), "f32r"
# (single-pass TF32-like; ~3e-4), "f32" (two-pass full fp32; ~7e-7).
import os as _os
MM_PREC = _os.environ.get("CIRC_MM_PREC", "bf16")
MM_DT = {"bf16": BF16, "f32r": F32R, "f32": F32}[MM_PREC]
N_WARM = int(_os.environ.get("CIRC_WARM", "10"))  # zero matmuls folded into stage 1


def _np_in(a):
    """Cast to the matmul precision; bf16 data is shipped packed in fp32
    words (DMA is element-rate-bound: 2-byte elements run at half rate)."""
    import ml_dtypes
    a = np.ascontiguousarray(np.asarray(a, dtype=np.float32))
    if MM_PREC != "bf16":
        return a
    bf = np.ascontiguousarray(a.astype(ml_dtypes.bfloat16))
    return bf.view(np.uint8).reshape(a.shape[0], -1).view(np.float32)

# Number of fp32 transport words per logical input element.
PACK = 2 if MM_PREC == "bf16" else 1
# Transport dtype: bf16 ships packed in fp32 words; f32/f32r ship natively.
TR_DT = F32 if MM_PREC == "bf16" else MM_DT
# Output transport: bf16 halves the store; f32/f32r debug modes store fp32.
OUT_DT = BF16 if MM_PREC == "bf16" else F32

XW = (D_IN + S) // PACK   # words per partition per j-chunk (kt | cc)
DW = 2 * B // PACK        # words per partition per k-block of (des^T|body^T)
GW = N // PACK            # words per partition per s-plane of G

# Stashed by kernel() for test harnesses that want profiling info.
LAST_RESULT = None

_nc_cache = {}


def _build_nc():
    """Build the (single-program) Bass module run on all 8 cores."""
    nc = bass.Bass(target_bir_lowering=True)

    # K^T and CC are packed together per j-chunk so each chunk DMA delivers
    # a self-sufficient unit of stage-1 work.  All inputs are host-packed
    # per SBUF partition: row p holds everything partition p receives.
    ktcc_q = [nc.declare_dram_parameter(f"ktcc{q}", [128, JC // NQ, XW], TR_DT, False)
              for q in range(NQ)]
    dbt_d = nc.declare_dram_parameter("dbt", [128, KB * DW], TR_DT, False)
    g_d = nc.declare_dram_parameter("g", [SH, 2 * GW], TR_DT, False)
    out_d = nc.declare_dram_parameter("out", [B, N], OUT_DT, isOutput=True)
    DEBUG = _os.environ.get("CIRC_DEBUG", "")
    kcdbg_d = (nc.declare_dram_parameter("kcdbg", [128, KB * S], F32, isOutput=True)
               if "kc" in DEBUG else None)
    dbdbg_d = (nc.declare_dram_parameter("dbdbg", [SH, 2 * 2 * B], F32, isOutput=True)
               if "db" in DEBUG else None)

    with tile.TileContext(nc) as tc:
        with (
            tc.tile_pool(name="main", bufs=1) as pool,
            tc.tile_pool(name="psum", bufs=1, space="PSUM") as pp,
        ):
            # ---- inputs -> SBUF ----
            # All input transfers ride ONE serial SP chain (a serial chain
            # pipelines; parallel channels all pay the full proxy latency).
            # dbt/g ride in the middle: late enough not to delay the first
            # K chunks, early enough that staging copies beat stage 2.
            dbg_dmas = []
            ktcc_sb = [pool.tile([128, JC // NQ, XW], TR_DT, tag=f"ktcc{q}", name=f"ktcc{q}")
                       for q in range(NQ)]
            dbt_raw = pool.tile([128, KB * DW], TR_DT, tag="dbtr", name="dbtr")
            g_raw = pool.tile([SH, 2 * GW], TR_DT, tag="gr", name="gr")
            in_dmas = [nc.sync.dma_start(ktcc_sb[q][:], ktcc_q[q][:, :, :])
                       for q in range(2)]
            in_dmas.append(nc.sync.dma_start(dbt_raw[:], dbt_d[:, :]))
            in_dmas.append(nc.sync.dma_start(g_raw[:], g_d[:, :]))
            in_dmas.extend(nc.sync.dma_start(ktcc_sb[q][:], ktcc_q[q][:, :, :])
                           for q in range(2, NQ))

            ktcc_v = [t.bitcast(MM_DT) for t in ktcc_sb]
            kt_sb = [ktcc_v[j // 2][:, j % 2, :D_IN] for j in range(JC)]
            cc_sb = [ktcc_v[j // 2][:, j % 2, D_IN:] for j in range(JC)]

            # Staging copies: DMA-sem -> Vector-sem so stage-2/4 matmuls
            # need only a single (Vector) wait.
            dbt_sb = pool.tile([128, KB, DW], TR_DT, tag="dbt", name="dbt")
            g_sb = pool.tile([SH, 2, GW], TR_DT, tag="g", name="g")
            nc.vector.tensor_copy(
                dbt_sb[:], dbt_raw[:].rearrange("p (kb w) -> p kb w", kb=KB))
            nc.vector.tensor_copy(
                g_sb[:], g_raw[:].rearrange("p (sb w) -> p sb w", sb=2))
            dbt_v = dbt_sb.bitcast(MM_DT)   # [128, KB, 2B]
            g_v = g_sb.bitcast(MM_DT)       # [SH, 2, N]

            # ---- PSUM: stage 1 in banks 0-1 (kb quads share a bank inside
            # one ordered chain), stage 2 banks 2-3, stage 4 banks 4-5.
            s1_ps = pp.tile([128, 2, 512], F32, tag="s1", name="s1")
            kc_ps = lambda kb: s1_ps[:, kb // 4, (kb % 4) * S:(kb % 4) * S + S]
            db_ps = pp.tile([SH, 2, 512], F32, tag="dbp", name="dbp")
            o_ps = [pp.tile([128, 512], F32, tag=f"op{h}", name=f"op{h}")
                    for h in range(2)]

            # ---- stage 1 (with folded warmup): KC[k,s] = sum_j KT[j,k]*CC[j,s]
            wz = pool.tile([128, 128 + S], BF16, tag="wz", name="wz")
            nc.gpsimd.memset(wz[:], 0.0)
            order = []  # explicit program-order edges (free: sync=False)
            for w in range(N_WARM):
                order.append(nc.tensor.matmul(
                    kc_ps(0), wz[:, :128], wz[:, 128:128 + S],
                    start=(w == 0), stop=False,
                    skip_group_check=True))
            for j in range(JC):
                for kb in range(KB):
                    # Bank clears: warmup mm 0 cleared bank 0; bank 1 is
                    # cleared by its first-ever matmul (j0, kb4).
                    order.append(nc.tensor.matmul(
                        kc_ps(kb),
                        kt_sb[j][:, kb * 128:(kb + 1) * 128],
                        cc_sb[j][:],
                        start=(j == 0 and (kb == 4 or (kb == 0 and N_WARM == 0))),
                        stop=(j == JC - 1),
                        skip_group_check=True,
                    ))
            for a, b_ in zip(order, order[1:]):
                add_dep_helper(b_.ins, a.ins, sync=False,
                               reason="stage1: keep bank-sharing chains ordered")

            # ---- drain stage-1 psum with ONE Vector copy ----
            # s1_ps viewed [128, 8, S] is exactly kb-major order.
            kc_sb = pool.tile([128, KB, S], MM_DT, tag="kc", name="kc")
            nc.vector.tensor_copy(
                kc_sb[:], s1_ps[:].rearrange("p b (r s) -> p (b r) s", r=4))
            if kcdbg_d is not None:
                kcf = pool.tile([128, KB * S], F32, tag="kcf", name="kcf")
                nc.vector.tensor_copy(kcf[:].rearrange("p (kb s) -> p kb s", kb=KB), kc_sb[:])
                dbg_dmas.append(nc.scalar.dma_start(kcdbg_d[:, :], kcf[:]))

            # ---- stage 2: DT/BT = KC^T @ (des^T|body^T) ----
            # cos rows -> chain [SH, 2B] in bank 2; sin rows -> bank 3.
            for kb in range(KB):
                for hh in range(2):
                    nc.tensor.matmul(
                        db_ps[:, hh, :2 * B],
                        kc_sb[:, kb, hh * SH:(hh + 1) * SH],
                        dbt_v[:, kb, :],
                        start=(kb == 0),
                        stop=(kb == KB - 1),
                    )

            # ---- stage 3: complex pointwise multiply ----
            # t01 = [Dr*Br, Dr*Bi], t23 = [Di*Bi, Di*Br]
            # Pr = t01[0] - t23[0],  Pi = t01[1] + t23[1]
            db_all = pool.tile([SH, 2, 2 * B], F32, tag="dball", name="dball")
            nc.vector.tensor_copy(db_all[:], db_ps[:, :, :2 * B])
            if dbdbg_d is not None:
                dbg_dmas.append(nc.gpsimd.dma_start(
                    dbdbg_d[:, :], db_all[:].rearrange("p a b -> p (a b)")))
            t01 = pool.tile([SH, 2, B], F32, tag="t01", name="t01")
            t23 = pool.tile([SH, 2, B], F32, tag="t23", name="t23")
            pt = pool.tile([SH, 2, B], MM_DT, tag="pt", name="pt")
            dr_b = db_all[:, 0, :B][:, None, :].to_broadcast((SH, 2, B))
            di_b = db_all[:, 1, :B][:, None, :].to_broadcast((SH, 2, B))
            nc.vector.tensor_mul(t01[:], dr_b, db_all[:, :, B:])
            nc.vector.tensor_mul(t23[:], di_b, db_all[:, ::-1, B:])
            nc.vector.tensor_sub(pt[:, 0, :], t01[:, 0, :], t23[:, 0, :])
            nc.gpsimd.tensor_add(pt[:, 1, :], t01[:, 1, :], t23[:, 1, :])

            # ---- stage 4: part = PT^T @ G; store each half as it drains ----
            out_sb = pool.tile([128, N], OUT_DT, tag="outsb", name="outsb")
            last_mm = None
            for h in range(2):
                for sb in range(2):
                    last_mm = nc.tensor.matmul(
                        o_ps[h][:],
                        pt[:, sb, :],
                        g_v[:, sb, h * 512:(h + 1) * 512],
                        start=(sb == 0),
                        stop=(sb == 1),
                    )
            cp0 = nc.vector.tensor_copy(out_sb[:, :512], o_ps[0][:])
            store_a = nc.sync.dma_start(out_d[:, :512], out_sb[:, :512])
            cp1 = nc.scalar.copy(out_sb[:, 512:], o_ps[1][:])
            store_b = nc.scalar.dma_start(out_d[:, 512:], out_sb[:, 512:])

            # TileContext's exit emits one tail Drain waiting on every
            # outstanding semaphore; walrus caps instructions at ONE sync
            # wait.  Pre-absorb every tick into SP's clock with a chain of
            # single-wait drains so the tail drain needs none.
            prev = None
            for dep in [*in_dmas, *dbg_dmas, store_a, store_b, last_mm, cp0, cp1]:
                dr = nc.sync.drain(fusable=False)
                add_dep_helper(dr.ins, dep.ins, sync=True,
                               reason="tail: absorb tick into SP clock")
                if prev is not None:
                    add_dep_helper(dr.ins, prev.ins, sync=False,
                                   reason="tail: keep drain chain ordered")
                prev = dr

    return nc


def _dft_constants():
    """Per-core forward (CC) and inverse (G) half-spectrum DFT matrices.

    Core c owns f in [64c+1, 64c+64].  Inverse weights: 4/N for paired
    frequencies 1..511, 2/N for the self-conjugate f=512.  (f=0 is the
    host-side DC correction.)
    """
    j = np.arange(N, dtype=np.float64)
    ccs, gs = [], []
    for c in range(N_CORES):
        f = np.arange(SH * c + 1, SH * c + SH + 1, dtype=np.float64)
        ang = 2.0 * np.pi * np.outer(j, f) / N          # (j, f)
        cc = np.concatenate([np.cos(ang), -np.sin(ang)], axis=1)   # (N, S)
        w = np.full(SH, 4.0 / N)
        if c == N_CORES - 1:
            w[-1] = 2.0 / N                              # f = 512
        angT = ang.T                                     # (f, k)
        gr = w[:, None] * np.cos(angT)
        gi = -w[:, None] * np.sin(angT)
        gmat = np.concatenate([gr, gi], axis=1)          # (SH, 2N): [cos|sin]
        ccs.append(np.ascontiguousarray(cc, dtype=np.float32))
        gs.append(np.ascontiguousarray(gmat, dtype=np.float32))
    return ccs, gs


def _partition_pack(a):
    """(R, W) with R = n*128 -> (128, n, W): row p = stack of chunk rows p."""
    r, w = a.shape
    n = r // 128
    return np.ascontiguousarray(a.reshape(n, 128, w).transpose(1, 0, 2))


def kernel(des, body, kernel):
    global LAST_RESULT
    des = np.asarray(des, dtype=np.float32)
    body = np.asarray(body, dtype=np.float32)
    K = np.asarray(kernel, dtype=np.float32)
    kt_np = K.T  # (j, k)
    dbt_np = _partition_pack(_np_in(np.concatenate(
        [des.T, body.T], axis=1))).reshape(128, KB * DW)
    ccs, gs = _dft_constants()
    in_maps = []
    for c in range(N_CORES):
        ktcc = _partition_pack(_np_in(np.concatenate([kt_np, ccs[c]], axis=1)))
        m = {f"ktcc{q}": np.ascontiguousarray(ktcc[:, 2 * q:2 * q + 2, :])
             for q in range(NQ)}
        m["dbt"] = dbt_np
        m["g"] = np.ascontiguousarray(_np_in(gs[c]))
        in_maps.append(m)

    if "nc" not in _nc_cache:
        _nc_cache["nc"] = _build_nc()
    nc = _nc_cache["nc"]

    res = run_bass_kernel_spmd(nc, in_maps, list(range(N_CORES)))
    LAST_RESULT = res
    out = np.zeros((B, N), dtype=np.float32)
    for r in res.results:
        out += np.asarray(r["out"], dtype=np.float32)
    # DC (f=0) correction: out[b, :] += (2/N) * (sum_j d)(sum_j b), a rank-1
    # term folded into the host unshard sum.
    kv = K.sum(axis=1)
    out += (2.0 / N) * ((des @ kv) * (body @ kv))[:, None]
    return out


# revision 30
# speedup vs baseline: 1.0114x; 1.0114x over previous
r"""Circulant layer kernel for Trainium2 (8 NeuronCores).

Math: reference computes mv1 + mv2 where
  mv1 = batch_circulant(b) @ d,  mv2 = batch_circulant(d) @ b,
with d = des @ K, b = body @ K.  Both are the circular convolution of d and b
(circular convolution is commutative), so  out = 2 * circconv(d, b).

circconv via DFT:  out = 2 * Re(IDFT(DFT(d) * DFT(b))).  d and b are REAL,
so the spectrum is conjugate-symmetric and only frequencies 0..512 are
needed; paired frequencies 1..511 carry weight 4/N in the inverse, the
self-conjugate f=512 carries 2/N, and the f=0 (DC) term is a rank-1
correction added on the host during the unshard sum.

Sharding: core c owns the 64 frequencies f in [64c+1, 64c+64] (core 7's
last is f=512, whose sin column is identically zero).  Per core:
  KC_c   = K @ CC_c            (1024k x 128s)   fused projection+fwd DFT
  DT_c   = KC_c^T @ des^T      (128s x 128b)    \  shares stationary weights
  BT_c   = KC_c^T @ body^T     (128s x 128b)    /
  PT_c   = complex-mult(DT_c, BT_c)             (64f x 2 x 128b)
  part_c = (PT_c^T @ G_c)                       (128b x 1024)  inverse DFT
Host sums the 8 partials and adds the DC term (unshard).

Key structural facts this implementation is built around:
- walrus allows ONE sync wait per instruction, so every consumer's
  dependencies must collapse onto a single producer engine.  DMA-fed
  operands consumed together with engine-produced data (dbt, g) are
  staged through Vector copies.
- a PSUM accumulation chain's first matmul clears has_written bits for
  its WHOLE 2KB bank, and the Tile scheduler reorders matmuls, so bank
  sharing is only safe inside ONE chain with explicit order edges.
  Stage 1 packs four kb regions per bank inside one ordered chain;
  stages 2/4 use fresh banks so no bank is ever re-read after a rewrite.
- stage 1 runs j-outer with all 8 accumulators live, so each arriving
  K chunk (4 chunked DMAs) is consumed immediately.
- PE warmup (HAM clock ramp) is folded into stage 1: zero matmuls
  accumulate into the kb0 region before the real contributions.
"""

import numpy as np

import concourse.bass as bass
import concourse.mybir as mybir
import concourse.tile as tile
from concourse.bass_utils import run_bass_kernel_spmd
from concourse.tile_rust import add_dep_helper

B = 128        # batch
D_IN = 1024    # input feature dim (contraction k)
N = 1024       # output feature dim (conv length j)
N_CORES = 8
SH = 64             # frequencies per core (complex, from the half spectrum)
S = 2 * SH          # freq slots per core: [0:SH]=real(cos), [SH:2SH]=imag(-sin)
JC = N // 128       # 8 j-chunks (contraction of stage 1)
KB = D_IN // 128    # 8 k-blocks (output partitions of stage 1)
QSPLIT = [1, 1, 2, 2, 2]   # j-chunks per ktcc DMA (small first chunk:
NQ = len(QSPLIT)           # stage 1 starts as early as possible)
QOFF = [sum(QSPLIT[:i]) for i in range(NQ)]

F32 = mybir.dt.float32
F32R = mybir.dt.float32r
BF16 = mybir.dt.bfloat16

# Matmul operand precision: "bf16" (fastest; ~5e-3 rel err), "f32r"
# (single-pass TF32-like; ~3e-4), "f32" (two-pass full fp32; ~7e-7).
import os as _os
MM_PREC = _os.environ.get("CIRC_MM_PREC", "bf16")
MM_DT = {"bf16": BF16, "f32r": F32R, "f32": F32}[MM_PREC]
N_WARM = int(_os.environ.get("CIRC_WARM", "10"))  # zero matmuls folded into stage 1


def _np_in(a):
    """Cast to the matmul precision; bf16 data is shipped packed in fp32
    words (DMA is element-rate-bound: 2-byte elements run at half rate)."""
    import ml_dtypes
    a = np.ascontiguousarray(np.asarray(a, dtype=np.float32))
    if MM_PREC != "bf16":
        return a
    bf = np.ascontiguousarray(a.astype(ml_dtypes.bfloat16))
    return bf.view(np.uint8).reshape(a.shape[0], -1).view(np.float32)

# Number of fp32 transport words per logical input element.
PACK = 2 if MM_PREC == "bf16" else 1
# Transport dtype: bf16 ships packed in fp32 words; f32/f32r ship natively.
TR_DT = F32 if MM_PREC == "bf16" else MM_DT
# Output transport: bf16 halves the store; f32/f32r debug modes store fp32.
OUT_DT = BF16 if MM_PREC == "bf16" else F32

XW = (D_IN + S) // PACK   # words per partition per j-chunk (kt | cc)
DW = 2 * B // PACK        # words per partition per k-block of (des^T|body^T)
GW = N // PACK            # words per partition per s-plane of G

# Stashed by kernel() for test harnesses that want profiling info.
LAST_RESULT = None

_nc_cache = {}


def _build_nc():
    """Build the (single-program) Bass module run on all 8 cores."""
    nc = bass.Bass(target_bir_lowering=True)

    # K^T and CC are packed together per j-chunk so each chunk DMA delivers
    # a self-sufficient unit of stage-1 work.  All inputs are host-packed
    # per SBUF partition: row p holds everything partition p receives.
    ktcc_q = [nc.declare_dram_parameter(f"ktcc{q}", [128, QSPLIT[q] * XW], TR_DT, False)
              for q in range(NQ)]
    aux_d = nc.declare_dram_parameter("aux", [128, KB * DW + 2 * GW], TR_DT, False)
    out_d = nc.declare_dram_parameter("out", [B, N], OUT_DT, isOutput=True)
    DEBUG = _os.environ.get("CIRC_DEBUG", "")
    kcdbg_d = (nc.declare_dram_parameter("kcdbg", [128, KB * S], F32, isOutput=True)
               if "kc" in DEBUG else None)
    dbdbg_d = (nc.declare_dram_parameter("dbdbg", [SH, 2 * 2 * B], F32, isOutput=True)
               if "db" in DEBUG else None)

    with tile.TileContext(nc) as tc:
        with (
            tc.tile_pool(name="main", bufs=1) as pool,
            tc.tile_pool(name="psum", bufs=1, space="PSUM") as pp,
        ):
            # ---- inputs -> SBUF ----
            # All input transfers ride ONE serial SP chain (a serial chain
            # pipelines; parallel channels all pay the full proxy latency).
            # dbt/g ride in the middle: late enough not to delay the first
            # K chunks, early enough that staging copies beat stage 2.
            dbg_dmas = []
            ktcc_sb = [pool.tile([128, QSPLIT[q] * XW], TR_DT, tag=f"ktcc{q}", name=f"ktcc{q}")
                       for q in range(NQ)]
            aux_raw = pool.tile([128, KB * DW + 2 * GW], TR_DT, tag="auxr", name="auxr")
            in_dmas = [nc.sync.dma_start(ktcc_sb[q][:], ktcc_q[q][:, :])
                       for q in range(NQ - 1)]
            in_dmas.append(nc.sync.dma_start(aux_raw[:], aux_d[:, :]))
            in_dmas.append(nc.sync.dma_start(ktcc_sb[NQ - 1][:], ktcc_q[NQ - 1][:, :]))

            # j-chunk views into the flat per-DMA tiles
            _jq = {}
            for q in range(NQ):
                v = ktcc_sb[q].bitcast(MM_DT).rearrange(
                    "p (c x) -> p c x", c=QSPLIT[q])
                for r in range(QSPLIT[q]):
                    _jq[QOFF[q] + r] = v[:, r, :]
            kt_sb = [_jq[j][:, :D_IN] for j in range(JC)]
            cc_sb = [_jq[j][:, D_IN:] for j in range(JC)]

            # Staging copies: DMA-sem -> Vector-sem so stage-2/4 matmuls
            # need only a single (Vector) wait.
            dbt_sb = pool.tile([128, KB, DW], TR_DT, tag="dbt", name="dbt")
            g_sb = pool.tile([SH, 2, GW], TR_DT, tag="g", name="g")
            nc.vector.tensor_copy(
                dbt_sb[:], aux_raw[:, :KB * DW].rearrange("p (kb w) -> p kb w", kb=KB))
            nc.vector.tensor_copy(
                g_sb[:], aux_raw[:SH, KB * DW:].rearrange("p (sb w) -> p sb w", sb=2))
            dbt_v = dbt_sb.bitcast(MM_DT)   # [128, KB, 2B]
            g_v = g_sb.bitcast(MM_DT)       # [SH, 2, N]

            # ---- PSUM: stage 1 in banks 0-1 (kb quads share a bank inside
            # one ordered chain), stage 2 banks 2-3, stage 4 banks 4-5.
            s1_ps = pp.tile([128, 2, 512], F32, tag="s1", name="s1")
            kc_ps = lambda kb: s1_ps[:, kb // 4, (kb % 4) * S:(kb % 4) * S + S]
            db_ps = pp.tile([SH, 2, 512], F32, tag="dbp", name="dbp")
            o_ps = [pp.tile([128, 512], F32, tag=f"op{h}", name=f"op{h}")
                    for h in range(2)]

            # ---- stage 1 (with folded warmup): KC[k,s] = sum_j KT[j,k]*CC[j,s]
            wz = pool.tile([128, 128 + S], BF16, tag="wz", name="wz")
            nc.gpsimd.memset(wz[:], 0.0)
            order = []  # explicit program-order edges (free: sync=False)
            for w in range(N_WARM):
                order.append(nc.tensor.matmul(
                    kc_ps(0), wz[:, :128], wz[:, 128:128 + S],
                    start=(w == 0), stop=False,
                    skip_group_check=True))
            for j in range(JC):
                for kb in range(KB):
                    # Bank clears: warmup mm 0 cleared bank 0; bank 1 is
                    # cleared by its first-ever matmul (j0, kb4).
                    order.append(nc.tensor.matmul(
                        kc_ps(kb),
                        kt_sb[j][:, kb * 128:(kb + 1) * 128],
                        cc_sb[j][:],
                        start=(j == 0 and (kb == 4 or (kb == 0 and N_WARM == 0))),
                        stop=(j == JC - 1),
                        skip_group_check=True,
                    ))
            for a, b_ in zip(order, order[1:]):
                add_dep_helper(b_.ins, a.ins, sync=False,
                               reason="stage1: keep bank-sharing chains ordered")

            # ---- drain stage-1 psum with ONE Vector copy ----
            # s1_ps viewed [128, 8, S] is exactly kb-major order.
            kc_sb = pool.tile([128, KB, S], MM_DT, tag="kc", name="kc")
            nc.vector.tensor_copy(
                kc_sb[:], s1_ps[:].rearrange("p b (r s) -> p (b r) s", r=4))
            if kcdbg_d is not None:
                kcf = pool.tile([128, KB * S], F32, tag="kcf", name="kcf")
                nc.vector.tensor_copy(kcf[:].rearrange("p (kb s) -> p kb s", kb=KB), kc_sb[:])
                dbg_dmas.append(nc.scalar.dma_start(kcdbg_d[:, :], kcf[:]))

            # ---- stage 2: DT/BT = KC^T @ (des^T|body^T) ----
            # cos rows -> chain [SH, 2B] in bank 2; sin rows -> bank 3.
            for kb in range(KB):
                for hh in range(2):
                    nc.tensor.matmul(
                        db_ps[:, hh, :2 * B],
                        kc_sb[:, kb, hh * SH:(hh + 1) * SH],
                        dbt_v[:, kb, :],
                        start=(kb == 0),
                        stop=(kb == KB - 1),
                    )

            # ---- stage 3: complex pointwise multiply ----
            # t01 = [Dr*Br, Dr*Bi], t23 = [Di*Bi, Di*Br]
            # Pr = t01[0] - t23[0],  Pi = t01[1] + t23[1]
            db_all = pool.tile([SH, 2, 2 * B], F32, tag="dball", name="dball")
            nc.vector.tensor_copy(db_all[:], db_ps[:, :, :2 * B])
            if dbdbg_d is not None:
                dbg_dmas.append(nc.gpsimd.dma_start(
                    dbdbg_d[:, :], db_all[:].rearrange("p a b -> p (a b)")))
            t01 = pool.tile([SH, 2, B], F32, tag="t01", name="t01")
            t23 = pool.tile([SH, 2, B], F32, tag="t23", name="t23")
            pt = pool.tile([SH, 2, B], MM_DT, tag="pt", name="pt")
            dr_b = db_all[:, 0, :B][:, None, :].to_broadcast((SH, 2, B))
            di_b = db_all[:, 1, :B][:, None, :].to_broadcast((SH, 2, B))
            nc.vector.tensor_mul(t01[:], dr_b, db_all[:, :, B:])
            nc.vector.tensor_mul(t23[:], di_b, db_all[:, ::-1, B:])
            nc.vector.tensor_sub(pt[:, 0, :], t01[:, 0, :], t23[:, 0, :])
            nc.gpsimd.tensor_add(pt[:, 1, :], t01[:, 1, :], t23[:, 1, :])

            # ---- stage 4: part = PT^T @ G; store each half as it drains ----
            out_sb = pool.tile([128, N], OUT_DT, tag="outsb", name="outsb")
            last_mm = None
            for h in range(2):
                for sb in range(2):
                    last_mm = nc.tensor.matmul(
                        o_ps[h][:],
                        pt[:, sb, :],
                        g_v[:, sb, h * 512:(h + 1) * 512],
                        start=(sb == 0),
                        stop=(sb == 1),
                    )
            cp0 = nc.vector.tensor_copy(out_sb[:, :512], o_ps[0][:])
            store_a = nc.sync.dma_start(out_d[:, :512], out_sb[:, :512])
            cp1 = nc.scalar.copy(out_sb[:, 512:], o_ps[1][:])
            store_b = nc.scalar.dma_start(out_d[:, 512:], out_sb[:, 512:])

            # TileContext's exit emits one tail Drain waiting on every
            # outstanding semaphore; walrus caps instructions at ONE sync
            # wait.  Pre-absorb every tick into SP's clock with a chain of
            # single-wait drains so the tail drain needs none.
            prev = None
            for dep in [*in_dmas, *dbg_dmas, store_a, store_b, last_mm, cp0, cp1]:
                dr = nc.sync.drain(fusable=False)
                add_dep_helper(dr.ins, dep.ins, sync=True,
                               reason="tail: absorb tick into SP clock")
                if prev is not None:
                    add_dep_helper(dr.ins, prev.ins, sync=False,
                                   reason="tail: keep drain chain ordered")
                prev = dr

    return nc


def _dft_constants():
    """Per-core forward (CC) and inverse (G) half-spectrum DFT matrices.

    Core c owns f in [64c+1, 64c+64].  Inverse weights: 4/N for paired
    frequencies 1..511, 2/N for the self-conjugate f=512.  (f=0 is the
    host-side DC correction.)
    """
    j = np.arange(N, dtype=np.float64)
    ccs, gs = [], []
    for c in range(N_CORES):
        f = np.arange(SH * c + 1, SH * c + SH + 1, dtype=np.float64)
        ang = 2.0 * np.pi * np.outer(j, f) / N          # (j, f)
        cc = np.concatenate([np.cos(ang), -np.sin(ang)], axis=1)   # (N, S)
        w = np.full(SH, 4.0 / N)
        if c == N_CORES - 1:
            w[-1] = 2.0 / N                              # f = 512
        angT = ang.T                                     # (f, k)
        gr = w[:, None] * np.cos(angT)
        gi = -w[:, None] * np.sin(angT)
        gmat = np.concatenate([gr, gi], axis=1)          # (SH, 2N): [cos|sin]
        ccs.append(np.ascontiguousarray(cc, dtype=np.float32))
        gs.append(np.ascontiguousarray(gmat, dtype=np.float32))
    return ccs, gs


def _partition_pack(a):
    """(R, W) with R = n*128 -> (128, n, W): row p = stack of chunk rows p."""
    r, w = a.shape
    n = r // 128
    return np.ascontiguousarray(a.reshape(n, 128, w).transpose(1, 0, 2))


def kernel(des, body, kernel):
    global LAST_RESULT
    des = np.asarray(des, dtype=np.float32)
    body = np.asarray(body, dtype=np.float32)
    K = np.asarray(kernel, dtype=np.float32)
    kt_np = K.T  # (j, k)
    dbt_np = _partition_pack(_np_in(np.concatenate(
        [des.T, body.T], axis=1))).reshape(128, KB * DW)
    ccs, gs = _dft_constants()
    in_maps = []
    for c in range(N_CORES):
        ktcc = _partition_pack(_np_in(np.concatenate([kt_np, ccs[c]], axis=1)))
        m = {f"ktcc{q}": np.ascontiguousarray(
                ktcc[:, QOFF[q]:QOFF[q] + QSPLIT[q], :]).reshape(128, QSPLIT[q] * XW)
             for q in range(NQ)}
        # aux = [dbt | g]: g is (SH, 2N) packed onto partitions 0..SH-1,
        # zero-padded to 128 partitions.
        g_pk = np.zeros((128, 2 * GW), dtype=np.float32)
        g_pk[:SH] = _np_in(gs[c])
        m["aux"] = np.ascontiguousarray(np.concatenate([dbt_np, g_pk], axis=1))
        in_maps.append(m)

    if "nc" not in _nc_cache:
        _nc_cache["nc"] = _build_nc()
    nc = _nc_cache["nc"]

    res = run_bass_kernel_spmd(nc, in_maps, list(range(N_CORES)))
    LAST_RESULT = res
    out = np.zeros((B, N), dtype=np.float32)
    for r in res.results:
        out += np.asarray(r["out"], dtype=np.float32)
    # DC (f=0) correction: out[b, :] += (2/N) * (sum_j d)(sum_j b), a rank-1
    # term folded into the host unshard sum.
    kv = K.sum(axis=1)
    out += (2.0 / N) * ((des @ kv) * (body @ kv))[:, None]
    return out


# revision 34
# speedup vs baseline: 1.0527x; 1.0408x over previous
r"""Circulant layer kernel for Trainium2 (8 NeuronCores).

Math: reference computes mv1 + mv2 where
  mv1 = batch_circulant(b) @ d,  mv2 = batch_circulant(d) @ b,
with d = des @ K, b = body @ K.  Both are the circular convolution of d and b
(circular convolution is commutative), so  out = 2 * circconv(d, b).

circconv via DFT:  out = 2 * Re(IDFT(DFT(d) * DFT(b))).  d and b are REAL,
so the spectrum is conjugate-symmetric and only frequencies 0..512 are
needed; paired frequencies 1..511 carry weight 4/N in the inverse, the
self-conjugate f=512 carries 2/N, and the f=0 (DC) term is a rank-1
correction added on the host during the unshard sum.

Sharding: core c owns the 64 frequencies f in [64c+1, 64c+64] (core 7's
last is f=512, whose sin column is identically zero).  Per core:
  KC_c   = K @ CC_c            (1024k x 128s)   fused projection+fwd DFT
  DT_c   = KC_c^T @ des^T      (128s x 128b)    \  shares stationary weights
  BT_c   = KC_c^T @ body^T     (128s x 128b)    /
  PT_c   = complex-mult(DT_c, BT_c)             (64f x 2 x 128b)
  part_c = (PT_c^T @ G_c)                       (128b x 1024)  inverse DFT
Host sums the 8 partials and adds the DC term (unshard).

Key structural facts this implementation is built around:
- walrus allows ONE sync wait per instruction, so every consumer's
  dependencies must collapse onto a single producer engine.  DMA-fed
  operands consumed together with engine-produced data (dbt, g) are
  staged through Vector copies.
- a PSUM accumulation chain's first matmul clears has_written bits for
  its WHOLE 2KB bank, and the Tile scheduler reorders matmuls, so bank
  sharing is only safe inside ONE chain with explicit order edges.
  Stage 1 packs four kb regions per bank inside one ordered chain;
  stages 2/4 use fresh banks so no bank is ever re-read after a rewrite.
- stage 1 runs j-outer with all 8 accumulators live, so each arriving
  K chunk (4 chunked DMAs) is consumed immediately.
- PE warmup (HAM clock ramp) is folded into stage 1: zero matmuls
  accumulate into the kb0 region before the real contributions.
"""

import numpy as np

import concourse.bass as bass
import concourse.mybir as mybir
import concourse.tile as tile
from concourse.bass_utils import run_bass_kernel_spmd
from concourse.tile_rust import add_dep_helper

B = 128        # batch
D_IN = 1024    # input feature dim (contraction k)
N = 1024       # output feature dim (conv length j)
N_CORES = 8
SH = 64             # frequencies per core (complex, from the half spectrum)
S = 2 * SH          # freq slots per core: [0:SH]=real(cos), [SH:2SH]=imag(-sin)
JC = N // 128       # 8 j-chunks (contraction of stage 1)
KB = D_IN // 128    # 8 k-blocks (output partitions of stage 1)
QSPLIT = [2, 2, 2, 1, 1]   # j-chunks per ktcc DMA (small LAST chunks:
NQ = len(QSPLIT)           # minimal stage-1 tail after the stream ends)
QOFF = [sum(QSPLIT[:i]) for i in range(NQ)]

F32 = mybir.dt.float32
F32R = mybir.dt.float32r
BF16 = mybir.dt.bfloat16

# Matmul operand precision: "bf16" (fastest; ~5e-3 rel err), "f32r"
# (single-pass TF32-like; ~3e-4), "f32" (two-pass full fp32; ~7e-7).
import os as _os
MM_PREC = _os.environ.get("CIRC_MM_PREC", "bf16")
MM_DT = {"bf16": BF16, "f32r": F32R, "f32": F32}[MM_PREC]
N_WARM = int(_os.environ.get("CIRC_WARM", "10"))  # zero matmuls folded into stage 1


def _np_in(a):
    """Cast to the matmul precision; bf16 data is shipped packed in fp32
    words (DMA is element-rate-bound: 2-byte elements run at half rate)."""
    import ml_dtypes
    a = np.ascontiguousarray(np.asarray(a, dtype=np.float32))
    if MM_PREC != "bf16":
        return a
    bf = np.ascontiguousarray(a.astype(ml_dtypes.bfloat16))
    return bf.view(np.uint8).reshape(a.shape[0], -1).view(np.float32)

# Number of fp32 transport words per logical input element.
PACK = 2 if MM_PREC == "bf16" else 1
# Transport dtype: bf16 ships packed in fp32 words; f32/f32r ship natively.
TR_DT = F32 if MM_PREC == "bf16" else MM_DT
# Output transport: bf16 halves the store; f32/f32r debug modes store fp32.
OUT_DT = BF16 if MM_PREC == "bf16" else F32

XW = (D_IN + S) // PACK   # words per partition per j-chunk (kt | cc)
DW = 2 * B // PACK        # words per partition per k-block of (des^T|body^T)
GW = N // PACK            # words per partition per s-plane of G

# Stashed by kernel() for test harnesses that want profiling info.
LAST_RESULT = None

_nc_cache = {}


def _build_nc():
    """Build the (single-program) Bass module run on all 8 cores."""
    nc = bass.Bass(target_bir_lowering=True)

    # K^T and CC are packed together per j-chunk so each chunk DMA delivers
    # a self-sufficient unit of stage-1 work.  All inputs are host-packed
    # per SBUF partition: row p holds everything partition p receives.
    # the LAST K chunk carries dbt appended per partition (one DMA, two tiles' worth)
    ktcc_q = [nc.declare_dram_parameter(
                  f"ktcc{q}",
                  [128, QSPLIT[q] * XW + (KB * DW if q == NQ - 1 else 0)],
                  TR_DT, False)
              for q in range(NQ)]
    g_d = nc.declare_dram_parameter("g", [SH, 2 * GW], TR_DT, False)
    out_d = nc.declare_dram_parameter("out", [B, N], OUT_DT, isOutput=True)
    DEBUG = _os.environ.get("CIRC_DEBUG", "")
    kcdbg_d = (nc.declare_dram_parameter("kcdbg", [128, KB * S], F32, isOutput=True)
               if "kc" in DEBUG else None)
    dbdbg_d = (nc.declare_dram_parameter("dbdbg", [SH, 2 * 2 * B], F32, isOutput=True)
               if "db" in DEBUG else None)

    with tile.TileContext(nc) as tc:
        with (
            tc.tile_pool(name="main", bufs=1) as pool,
            tc.tile_pool(name="psum", bufs=1, space="PSUM") as pp,
        ):
            # ---- inputs -> SBUF ----
            # All input transfers ride ONE serial SP chain (a serial chain
            # pipelines; parallel channels all pay the full proxy latency).
            # dbt/g ride in the middle: late enough not to delay the first
            # K chunks, early enough that staging copies beat stage 2.
            dbg_dmas = []
            ktcc_sb = [pool.tile([128, QSPLIT[q] * XW + (KB * DW if q == NQ - 1 else 0)],
                                 TR_DT, tag=f"ktcc{q}", name=f"ktcc{q}")
                       for q in range(NQ)]
            g_raw = pool.tile([SH, 2 * GW], TR_DT, tag="gr", name="gr")
            in_dmas = [nc.sync.dma_start(ktcc_sb[q][:], ktcc_q[q][:, :])
                       for q in range(NQ)]
            in_dmas.append(nc.sync.dma_start(g_raw[:], g_d[:, :]))

            # j-chunk views into the flat per-DMA tiles
            _jq = {}
            for q in range(NQ):
                kpart = ktcc_sb[q][:, :QSPLIT[q] * XW] if q == NQ - 1 else ktcc_sb[q][:]
                v = kpart.bitcast(MM_DT).rearrange(
                    "p (c x) -> p c x", c=QSPLIT[q])
                for r in range(QSPLIT[q]):
                    _jq[QOFF[q] + r] = v[:, r, :]
            kt_sb = [_jq[j][:, :D_IN] for j in range(JC)]
            cc_sb = [_jq[j][:, D_IN:] for j in range(JC)]

            # Staging copies: DMA-sem -> Vector-sem so stage-2/4 matmuls
            # need only a single (Vector) wait.
            dbt_sb = pool.tile([128, KB, DW], TR_DT, tag="dbt", name="dbt")
            g_sb = pool.tile([SH, 2, GW], TR_DT, tag="g", name="g")
            nc.vector.tensor_copy(
                dbt_sb[:], ktcc_sb[NQ - 1][:, QSPLIT[NQ - 1] * XW:]
                .rearrange("p (kb w) -> p kb w", kb=KB))
            nc.vector.tensor_copy(
                g_sb[:], g_raw[:].rearrange("p (sb w) -> p sb w", sb=2))
            dbt_v = dbt_sb.bitcast(MM_DT)   # [128, KB, 2B]
            g_v = g_sb.bitcast(MM_DT)       # [SH, 2, N]

            # ---- PSUM: stage 1 in banks 0-1 (kb quads share a bank inside
            # one ordered chain), stage 2 banks 2-3, stage 4 banks 4-5.
            s1_ps = pp.tile([128, 2, 512], F32, tag="s1", name="s1")
            kc_ps = lambda kb: s1_ps[:, kb // 4, (kb % 4) * S:(kb % 4) * S + S]
            db_ps = pp.tile([SH, 2, 512], F32, tag="dbp", name="dbp")
            o_ps = [pp.tile([128, 512], F32, tag=f"op{h}", name=f"op{h}")
                    for h in range(2)]

            # ---- stage 1 (with folded warmup): KC[k,s] = sum_j KT[j,k]*CC[j,s]
            wz = pool.tile([128, 128 + S], BF16, tag="wz", name="wz")
            nc.gpsimd.memset(wz[:], 0.0)
            order = []  # explicit program-order edges (free: sync=False)
            for w in range(N_WARM):
                order.append(nc.tensor.matmul(
                    kc_ps(0), wz[:, :128], wz[:, 128:128 + S],
                    start=(w == 0), stop=False,
                    skip_group_check=True))
            for j in range(JC):
                for kb in range(KB):
                    # Bank clears: warmup mm 0 cleared bank 0; bank 1 is
                    # cleared by its first-ever matmul (j0, kb4).
                    order.append(nc.tensor.matmul(
                        kc_ps(kb),
                        kt_sb[j][:, kb * 128:(kb + 1) * 128],
                        cc_sb[j][:],
                        start=(j == 0 and (kb == 4 or (kb == 0 and N_WARM == 0))),
                        stop=(j == JC - 1),
                        skip_group_check=True,
                    ))
            for a, b_ in zip(order, order[1:]):
                add_dep_helper(b_.ins, a.ins, sync=False,
                               reason="stage1: keep bank-sharing chains ordered")

            # ---- drain stage-1 psum with ONE Vector copy ----
            # s1_ps viewed [128, 8, S] is exactly kb-major order.
            kc_sb = pool.tile([128, KB, S], MM_DT, tag="kc", name="kc")
            nc.vector.tensor_copy(
                kc_sb[:, 0:4, :], s1_ps[:, 0, :].rearrange("p (r s) -> p r s", r=4))
            nc.vector.tensor_copy(
                kc_sb[:, 4:8, :], s1_ps[:, 1, :].rearrange("p (r s) -> p r s", r=4))
            if kcdbg_d is not None:
                kcf = pool.tile([128, KB * S], F32, tag="kcf", name="kcf")
                nc.vector.tensor_copy(kcf[:].rearrange("p (kb s) -> p kb s", kb=KB), kc_sb[:])
                dbg_dmas.append(nc.scalar.dma_start(kcdbg_d[:, :], kcf[:]))

            # ---- stage 2: DT/BT = KC^T @ (des^T|body^T) ----
            # cos rows -> chain [SH, 2B] in bank 2; sin rows -> bank 3.
            for kb in range(KB):
                for hh in range(2):
                    nc.tensor.matmul(
                        db_ps[:, hh, :2 * B],
                        kc_sb[:, kb, hh * SH:(hh + 1) * SH],
                        dbt_v[:, kb, :],
                        start=(kb == 0),
                        stop=(kb == KB - 1),
                    )

            # ---- stage 3: complex pointwise multiply ----
            # t01 = [Dr*Br, Dr*Bi], t23 = [Di*Bi, Di*Br]
            # Pr = t01[0] - t23[0],  Pi = t01[1] + t23[1]
            dd = pool.tile([SH, 2, B], F32, tag="dd", name="dd")
            nc.vector.tensor_copy(dd[:], db_ps[:, :, :B])
            if dbdbg_d is not None:
                dball = pool.tile([SH, 2, 2 * B], F32, tag="dball", name="dball")
                nc.vector.tensor_copy(dball[:], db_ps[:, :, :2 * B])
                dbg_dmas.append(nc.gpsimd.dma_start(
                    dbdbg_d[:, :], dball[:].rearrange("p a b -> p (a b)")))
            t01 = pool.tile([SH, 2, B], F32, tag="t01", name="t01")
            t23 = pool.tile([SH, 2, B], F32, tag="t23", name="t23")
            pt = pool.tile([SH, 2, B], MM_DT, tag="pt", name="pt")
            dr_b = dd[:, 0, :][:, None, :].to_broadcast((SH, 2, B))
            di_b = dd[:, 1, :][:, None, :].to_broadcast((SH, 2, B))
            nc.vector.tensor_mul(t01[:], dr_b, db_ps[:, :, B:2 * B])
            nc.vector.tensor_mul(t23[:], di_b, db_ps[:, ::-1, B:2 * B])
            nc.vector.tensor_sub(pt[:, 0, :], t01[:, 0, :], t23[:, 0, :])
            nc.gpsimd.tensor_add(pt[:, 1, :], t01[:, 1, :], t23[:, 1, :])

            # ---- stage 4: part = PT^T @ G; store each half as it drains ----
            out_sb = pool.tile([128, N], OUT_DT, tag="outsb", name="outsb")
            last_mm = None
            for h in range(2):
                for sb in range(2):
                    last_mm = nc.tensor.matmul(
                        o_ps[h][:],
                        pt[:, sb, :],
                        g_v[:, sb, h * 512:(h + 1) * 512],
                        start=(sb == 0),
                        stop=(sb == 1),
                    )
            cp0 = nc.vector.tensor_copy(out_sb[:, :512], o_ps[0][:])
            store_a = nc.sync.dma_start(out_d[:, :512], out_sb[:, :512])
            cp1 = nc.scalar.copy(out_sb[:, 512:], o_ps[1][:])
            store_b = nc.scalar.dma_start(out_d[:, 512:], out_sb[:, 512:])

            # TileContext's exit emits one tail Drain waiting on every
            # outstanding semaphore; walrus caps instructions at ONE sync
            # wait.  Pre-absorb every tick into SP's clock with a chain of
            # single-wait drains so the tail drain needs none.
            prev = None
            for dep in [*in_dmas, *dbg_dmas, store_a, store_b, last_mm, cp0, cp1]:
                dr = nc.sync.drain(fusable=False)
                add_dep_helper(dr.ins, dep.ins, sync=True,
                               reason="tail: absorb tick into SP clock")
                if prev is not None:
                    add_dep_helper(dr.ins, prev.ins, sync=False,
                                   reason="tail: keep drain chain ordered")
                prev = dr

    return nc


def _dft_constants():
    """Per-core forward (CC) and inverse (G) half-spectrum DFT matrices.

    Core c owns f in [64c+1, 64c+64].  Inverse weights: 4/N for paired
    frequencies 1..511, 2/N for the self-conjugate f=512.  (f=0 is the
    host-side DC correction.)
    """
    j = np.arange(N, dtype=np.float64)
    ccs, gs = [], []
    for c in range(N_CORES):
        f = np.arange(SH * c + 1, SH * c + SH + 1, dtype=np.float64)
        ang = 2.0 * np.pi * np.outer(j, f) / N          # (j, f)
        cc = np.concatenate([np.cos(ang), -np.sin(ang)], axis=1)   # (N, S)
        w = np.full(SH, 4.0 / N)
        if c == N_CORES - 1:
            w[-1] = 2.0 / N                              # f = 512
        angT = ang.T                                     # (f, k)
        gr = w[:, None] * np.cos(angT)
        gi = -w[:, None] * np.sin(angT)
        gmat = np.concatenate([gr, gi], axis=1)          # (SH, 2N): [cos|sin]
        ccs.append(np.ascontiguousarray(cc, dtype=np.float32))
        gs.append(np.ascontiguousarray(gmat, dtype=np.float32))
    return ccs, gs


def _partition_pack(a):
    """(R, W) with R = n*128 -> (128, n, W): row p = stack of chunk rows p."""
    r, w = a.shape
    n = r // 128
    return np.ascontiguousarray(a.reshape(n, 128, w).transpose(1, 0, 2))


def kernel(des, body, kernel):
    global LAST_RESULT
    des = np.asarray(des, dtype=np.float32)
    body = np.asarray(body, dtype=np.float32)
    K = np.asarray(kernel, dtype=np.float32)
    kt_np = K.T  # (j, k)
    dbt_np = _partition_pack(_np_in(np.concatenate(
        [des.T, body.T], axis=1))).reshape(128, KB * DW)
    ccs, gs = _dft_constants()
    in_maps = []
    for c in range(N_CORES):
        ktcc = _partition_pack(_np_in(np.concatenate([kt_np, ccs[c]], axis=1)))
        m = {f"ktcc{q}": np.ascontiguousarray(
                ktcc[:, QOFF[q]:QOFF[q] + QSPLIT[q], :]).reshape(128, QSPLIT[q] * XW)
             for q in range(NQ)}
        m[f"ktcc{NQ - 1}"] = np.ascontiguousarray(
            np.concatenate([m[f"ktcc{NQ - 1}"], dbt_np], axis=1))
        m["g"] = np.ascontiguousarray(_np_in(gs[c]))
        in_maps.append(m)

    if "nc" not in _nc_cache:
        _nc_cache["nc"] = _build_nc()
    nc = _nc_cache["nc"]

    res = run_bass_kernel_spmd(nc, in_maps, list(range(N_CORES)))
    LAST_RESULT = res
    out = np.zeros((B, N), dtype=np.float32)
    for r in res.results:
        out += np.asarray(r["out"], dtype=np.float32)
    # DC (f=0) correction: out[b, :] += (2/N) * (sum_j d)(sum_j b), a rank-1
    # term folded into the host unshard sum.
    kv = K.sum(axis=1)
    out += (2.0 / N) * ((des @ kv) * (body @ kv))[:, None]
    return out


# revision 35
# speedup vs baseline: 1.0740x; 1.0203x over previous
r"""Circulant layer kernel for Trainium2 (8 NeuronCores).

Math: reference computes mv1 + mv2 where
  mv1 = batch_circulant(b) @ d,  mv2 = batch_circulant(d) @ b,
with d = des @ K, b = body @ K.  Both are the circular convolution of d and b
(circular convolution is commutative), so  out = 2 * circconv(d, b).

circconv via DFT:  out = 2 * Re(IDFT(DFT(d) * DFT(b))).  d and b are REAL,
so the spectrum is conjugate-symmetric and only frequencies 0..512 are
needed; paired frequencies 1..511 carry weight 4/N in the inverse, the
self-conjugate f=512 carries 2/N, and the f=0 (DC) term is a rank-1
correction added on the host during the unshard sum.

Sharding: core c owns the 64 frequencies f in [64c+1, 64c+64] (core 7's
last is f=512, whose sin column is identically zero).  Per core:
  KC_c   = K @ CC_c            (1024k x 128s)   fused projection+fwd DFT
  DT_c   = KC_c^T @ des^T      (128s x 128b)    \  shares stationary weights
  BT_c   = KC_c^T @ body^T     (128s x 128b)    /
  PT_c   = complex-mult(DT_c, BT_c)             (64f x 2 x 128b)
  part_c = (PT_c^T @ G_c)                       (128b x 1024)  inverse DFT
Host sums the 8 partials and adds the DC term (unshard).

Key structural facts this implementation is built around:
- walrus allows ONE sync wait per instruction, so every consumer's
  dependencies must collapse onto a single producer engine.  DMA-fed
  operands consumed together with engine-produced data (dbt, g) are
  staged through Vector copies.
- a PSUM accumulation chain's first matmul clears has_written bits for
  its WHOLE 2KB bank, and the Tile scheduler reorders matmuls, so bank
  sharing is only safe inside ONE chain with explicit order edges.
  Stage 1 packs four kb regions per bank inside one ordered chain;
  stages 2/4 use fresh banks so no bank is ever re-read after a rewrite.
- stage 1 runs j-outer with all 8 accumulators live, so each arriving
  K chunk (4 chunked DMAs) is consumed immediately.
- PE warmup (HAM clock ramp) is folded into stage 1: zero matmuls
  accumulate into the kb0 region before the real contributions.
"""

import numpy as np

import concourse.bass as bass
import concourse.mybir as mybir
import concourse.tile as tile
from concourse.bass_utils import run_bass_kernel_spmd
from concourse.tile_rust import add_dep_helper

B = 128        # batch
D_IN = 1024    # input feature dim (contraction k)
N = 1024       # output feature dim (conv length j)
N_CORES = 8
SH = 64             # frequencies per core (complex, from the half spectrum)
S = 2 * SH          # freq slots per core: [0:SH]=real(cos), [SH:2SH]=imag(-sin)
JC = N // 128       # 8 j-chunks (contraction of stage 1)
KB = D_IN // 128    # 8 k-blocks (output partitions of stage 1)
QSPLIT = [2, 2, 2, 1, 1]   # j-chunks per ktcc DMA (small LAST chunks:
NQ = len(QSPLIT)           # minimal stage-1 tail after the stream ends)
QOFF = [sum(QSPLIT[:i]) for i in range(NQ)]

F32 = mybir.dt.float32
F32R = mybir.dt.float32r
BF16 = mybir.dt.bfloat16

# Matmul operand precision: "bf16" (fastest; ~5e-3 rel err), "f32r"
# (single-pass TF32-like; ~3e-4), "f32" (two-pass full fp32; ~7e-7).
import os as _os
MM_PREC = _os.environ.get("CIRC_MM_PREC", "bf16")
MM_DT = {"bf16": BF16, "f32r": F32R, "f32": F32}[MM_PREC]
N_WARM = int(_os.environ.get("CIRC_WARM", "10"))  # zero matmuls folded into stage 1


def _np_in(a):
    """Cast to the matmul precision; bf16 data is shipped packed in fp32
    words (DMA is element-rate-bound: 2-byte elements run at half rate)."""
    import ml_dtypes
    a = np.ascontiguousarray(np.asarray(a, dtype=np.float32))
    if MM_PREC != "bf16":
        return a
    bf = np.ascontiguousarray(a.astype(ml_dtypes.bfloat16))
    return bf.view(np.uint8).reshape(a.shape[0], -1).view(np.float32)

# Number of fp32 transport words per logical input element.
PACK = 2 if MM_PREC == "bf16" else 1
# Transport dtype: bf16 ships packed in fp32 words; f32/f32r ship natively.
TR_DT = F32 if MM_PREC == "bf16" else MM_DT
# Output transport: bf16 halves the store; f32/f32r debug modes store fp32.
OUT_DT = BF16 if MM_PREC == "bf16" else F32

XW = (D_IN + S) // PACK   # words per partition per j-chunk (kt | cc)
DW = 2 * B // PACK        # words per partition per k-block of (des^T|body^T)
GW = N // PACK            # words per partition per s-plane of G

# Stashed by kernel() for test harnesses that want profiling info.
LAST_RESULT = None

_nc_cache = {}


def _build_nc():
    """Build the (single-program) Bass module run on all 8 cores."""
    nc = bass.Bass(target_bir_lowering=True)

    # K^T and CC are packed together per j-chunk so each chunk DMA delivers
    # a self-sufficient unit of stage-1 work.  All inputs are host-packed
    # per SBUF partition: row p holds everything partition p receives.
    # chunk 0 carries dbt appended per partition; chunk 1 carries g rows
    # (on its first SH partitions) -- their staging copies then run early,
    # while Vector is otherwise idle.
    EXTRA = [KB * DW if q == 0 else (2 * GW if q == 1 else 0) for q in range(NQ)]
    ktcc_q = [nc.declare_dram_parameter(
                  f"ktcc{q}", [128, QSPLIT[q] * XW + EXTRA[q]], TR_DT, False)
              for q in range(NQ)]
    out_d = nc.declare_dram_parameter("out", [B, N], OUT_DT, isOutput=True)
    DEBUG = _os.environ.get("CIRC_DEBUG", "")
    kcdbg_d = (nc.declare_dram_parameter("kcdbg", [128, KB * S], F32, isOutput=True)
               if "kc" in DEBUG else None)
    dbdbg_d = (nc.declare_dram_parameter("dbdbg", [SH, 2 * 2 * B], F32, isOutput=True)
               if "db" in DEBUG else None)

    with tile.TileContext(nc) as tc:
        with (
            tc.tile_pool(name="main", bufs=1) as pool,
            tc.tile_pool(name="psum", bufs=1, space="PSUM") as pp,
        ):
            # ---- inputs -> SBUF ----
            # All input transfers ride ONE serial SP chain (a serial chain
            # pipelines; parallel channels all pay the full proxy latency).
            # dbt/g ride in the middle: late enough not to delay the first
            # K chunks, early enough that staging copies beat stage 2.
            dbg_dmas = []
            ktcc_sb = [pool.tile([128, QSPLIT[q] * XW + EXTRA[q]],
                                 TR_DT, tag=f"ktcc{q}", name=f"ktcc{q}")
                       for q in range(NQ)]
            in_dmas = [nc.sync.dma_start(ktcc_sb[q][:], ktcc_q[q][:, :])
                       for q in range(NQ)]

            # j-chunk views into the flat per-DMA tiles
            _jq = {}
            for q in range(NQ):
                kpart = ktcc_sb[q][:, :QSPLIT[q] * XW]
                v = kpart.bitcast(MM_DT).rearrange(
                    "p (c x) -> p c x", c=QSPLIT[q])
                for r in range(QSPLIT[q]):
                    _jq[QOFF[q] + r] = v[:, r, :]
            kt_sb = [_jq[j][:, :D_IN] for j in range(JC)]
            cc_sb = [_jq[j][:, D_IN:] for j in range(JC)]

            # Staging copies: DMA-sem -> Vector-sem so stage-2/4 matmuls
            # need only a single (Vector) wait.
            dbt_sb = pool.tile([128, KB, DW], TR_DT, tag="dbt", name="dbt")
            g_sb = pool.tile([SH, 2, GW], TR_DT, tag="g", name="g")
            nc.vector.tensor_copy(
                dbt_sb[:], ktcc_sb[0][:, QSPLIT[0] * XW:]
                .rearrange("p (kb w) -> p kb w", kb=KB))
            nc.vector.tensor_copy(
                g_sb[:], ktcc_sb[1][:SH, QSPLIT[1] * XW:]
                .rearrange("p (sb w) -> p sb w", sb=2))
            dbt_v = dbt_sb.bitcast(MM_DT)   # [128, KB, 2B]
            g_v = g_sb.bitcast(MM_DT)       # [SH, 2, N]

            # ---- PSUM: stage 1 in banks 0-1 (kb quads share a bank inside
            # one ordered chain), stage 2 banks 2-3, stage 4 banks 4-5.
            s1_ps = pp.tile([128, 2, 512], F32, tag="s1", name="s1")
            kc_ps = lambda kb: s1_ps[:, kb // 4, (kb % 4) * S:(kb % 4) * S + S]
            db_ps = pp.tile([SH, 2, 512], F32, tag="dbp", name="dbp")
            o_ps = [pp.tile([128, 512], F32, tag=f"op{h}", name=f"op{h}")
                    for h in range(2)]

            # ---- stage 1 (with folded warmup): KC[k,s] = sum_j KT[j,k]*CC[j,s]
            wz = pool.tile([128, 128 + S], BF16, tag="wz", name="wz")
            nc.gpsimd.memset(wz[:], 0.0)
            order = []  # explicit program-order edges (free: sync=False)
            for w in range(N_WARM):
                order.append(nc.tensor.matmul(
                    kc_ps(0), wz[:, :128], wz[:, 128:128 + S],
                    start=(w == 0), stop=False,
                    skip_group_check=True))
            for j in range(JC):
                for kb in range(KB):
                    # Bank clears: warmup mm 0 cleared bank 0; bank 1 is
                    # cleared by its first-ever matmul (j0, kb4).
                    order.append(nc.tensor.matmul(
                        kc_ps(kb),
                        kt_sb[j][:, kb * 128:(kb + 1) * 128],
                        cc_sb[j][:],
                        start=(j == 0 and (kb == 4 or (kb == 0 and N_WARM == 0))),
                        stop=(j == JC - 1),
                        skip_group_check=True,
                    ))
            for a, b_ in zip(order, order[1:]):
                add_dep_helper(b_.ins, a.ins, sync=False,
                               reason="stage1: keep bank-sharing chains ordered")

            # ---- drain stage-1 psum with ONE Vector copy ----
            # s1_ps viewed [128, 8, S] is exactly kb-major order.
            kc_sb = pool.tile([128, KB, S], MM_DT, tag="kc", name="kc")
            nc.vector.tensor_copy(
                kc_sb[:, 0:4, :], s1_ps[:, 0, :].rearrange("p (r s) -> p r s", r=4))
            nc.vector.tensor_copy(
                kc_sb[:, 4:8, :], s1_ps[:, 1, :].rearrange("p (r s) -> p r s", r=4))
            if kcdbg_d is not None:
                kcf = pool.tile([128, KB * S], F32, tag="kcf", name="kcf")
                nc.vector.tensor_copy(kcf[:].rearrange("p (kb s) -> p kb s", kb=KB), kc_sb[:])
                dbg_dmas.append(nc.scalar.dma_start(kcdbg_d[:, :], kcf[:]))

            # ---- stage 2: DT/BT = KC^T @ (des^T|body^T) ----
            # cos rows -> chain [SH, 2B] in bank 2; sin rows -> bank 3.
            for kb in range(KB):
                for hh in range(2):
                    nc.tensor.matmul(
                        db_ps[:, hh, :2 * B],
                        kc_sb[:, kb, hh * SH:(hh + 1) * SH],
                        dbt_v[:, kb, :],
                        start=(kb == 0),
                        stop=(kb == KB - 1),
                    )

            # ---- stage 3: complex pointwise multiply ----
            # t01 = [Dr*Br, Dr*Bi], t23 = [Di*Bi, Di*Br]
            # Pr = t01[0] - t23[0],  Pi = t01[1] + t23[1]
            dd = pool.tile([SH, 2, B], F32, tag="dd", name="dd")
            nc.vector.tensor_copy(dd[:], db_ps[:, :, :B])
            if dbdbg_d is not None:
                dball = pool.tile([SH, 2, 2 * B], F32, tag="dball", name="dball")
                nc.vector.tensor_copy(dball[:], db_ps[:, :, :2 * B])
                dbg_dmas.append(nc.gpsimd.dma_start(
                    dbdbg_d[:, :], dball[:].rearrange("p a b -> p (a b)")))
            t01 = pool.tile([SH, 2, B], F32, tag="t01", name="t01")
            t23 = pool.tile([SH, 2, B], F32, tag="t23", name="t23")
            pt = pool.tile([SH, 2, B], MM_DT, tag="pt", name="pt")
            dr_b = dd[:, 0, :][:, None, :].to_broadcast((SH, 2, B))
            di_b = dd[:, 1, :][:, None, :].to_broadcast((SH, 2, B))
            nc.vector.tensor_mul(t01[:], dr_b, db_ps[:, :, B:2 * B])
            nc.vector.tensor_mul(t23[:], di_b, db_ps[:, ::-1, B:2 * B])
            nc.vector.tensor_sub(pt[:, 0, :], t01[:, 0, :], t23[:, 0, :])
            nc.gpsimd.tensor_add(pt[:, 1, :], t01[:, 1, :], t23[:, 1, :])

            # ---- stage 4: part = PT^T @ G; store each half as it drains ----
            out_sb = pool.tile([128, N], OUT_DT, tag="outsb", name="outsb")
            last_mm = None
            for h in range(2):
                for sb in range(2):
                    last_mm = nc.tensor.matmul(
                        o_ps[h][:],
                        pt[:, sb, :],
                        g_v[:, sb, h * 512:(h + 1) * 512],
                        start=(sb == 0),
                        stop=(sb == 1),
                    )
            cp0 = nc.vector.tensor_copy(out_sb[:, :512], o_ps[0][:])
            store_a = nc.sync.dma_start(out_d[:, :512], out_sb[:, :512])
            cp1 = nc.scalar.copy(out_sb[:, 512:], o_ps[1][:])
            store_b = nc.scalar.dma_start(out_d[:, 512:], out_sb[:, 512:])

            # TileContext's exit emits one tail Drain waiting on every
            # outstanding semaphore; walrus caps instructions at ONE sync
            # wait.  Pre-absorb every tick into SP's clock with a chain of
            # single-wait drains so the tail drain needs none.
            prev = None
            for dep in [*in_dmas, *dbg_dmas, store_a, store_b, last_mm, cp0, cp1]:
                dr = nc.sync.drain(fusable=False)
                add_dep_helper(dr.ins, dep.ins, sync=True,
                               reason="tail: absorb tick into SP clock")
                if prev is not None:
                    add_dep_helper(dr.ins, prev.ins, sync=False,
                                   reason="tail: keep drain chain ordered")
                prev = dr

    return nc


def _dft_constants():
    """Per-core forward (CC) and inverse (G) half-spectrum DFT matrices.

    Core c owns f in [64c+1, 64c+64].  Inverse weights: 4/N for paired
    frequencies 1..511, 2/N for the self-conjugate f=512.  (f=0 is the
    host-side DC correction.)
    """
    j = np.arange(N, dtype=np.float64)
    ccs, gs = [], []
    for c in range(N_CORES):
        f = np.arange(SH * c + 1, SH * c + SH + 1, dtype=np.float64)
        ang = 2.0 * np.pi * np.outer(j, f) / N          # (j, f)
        cc = np.concatenate([np.cos(ang), -np.sin(ang)], axis=1)   # (N, S)
        w = np.full(SH, 4.0 / N)
        if c == N_CORES - 1:
            w[-1] = 2.0 / N                              # f = 512
        angT = ang.T                                     # (f, k)
        gr = w[:, None] * np.cos(angT)
        gi = -w[:, None] * np.sin(angT)
        gmat = np.concatenate([gr, gi], axis=1)          # (SH, 2N): [cos|sin]
        ccs.append(np.ascontiguousarray(cc, dtype=np.float32))
        gs.append(np.ascontiguousarray(gmat, dtype=np.float32))
    return ccs, gs


def _partition_pack(a):
    """(R, W) with R = n*128 -> (128, n, W): row p = stack of chunk rows p."""
    r, w = a.shape
    n = r // 128
    return np.ascontiguousarray(a.reshape(n, 128, w).transpose(1, 0, 2))


def kernel(des, body, kernel):
    global LAST_RESULT
    des = np.asarray(des, dtype=np.float32)
    body = np.asarray(body, dtype=np.float32)
    K = np.asarray(kernel, dtype=np.float32)
    kt_np = K.T  # (j, k)
    dbt_np = _partition_pack(_np_in(np.concatenate(
        [des.T, body.T], axis=1))).reshape(128, KB * DW)
    ccs, gs = _dft_constants()
    in_maps = []
    for c in range(N_CORES):
        ktcc = _partition_pack(_np_in(np.concatenate([kt_np, ccs[c]], axis=1)))
        m = {f"ktcc{q}": np.ascontiguousarray(
                ktcc[:, QOFF[q]:QOFF[q] + QSPLIT[q], :]).reshape(128, QSPLIT[q] * XW)
             for q in range(NQ)}
        m["ktcc0"] = np.ascontiguousarray(np.concatenate([m["ktcc0"], dbt_np], axis=1))
        g_pad = np.zeros((128, 2 * GW), dtype=np.float32)
        g_pad[:SH] = _np_in(gs[c])
        m["ktcc1"] = np.ascontiguousarray(np.concatenate([m["ktcc1"], g_pad], axis=1))
        in_maps.append(m)

    if "nc" not in _nc_cache:
        _nc_cache["nc"] = _build_nc()
    nc = _nc_cache["nc"]

    res = run_bass_kernel_spmd(nc, in_maps, list(range(N_CORES)))
    LAST_RESULT = res
    out = np.zeros((B, N), dtype=np.float32)
    for r in res.results:
        out += np.asarray(r["out"], dtype=np.float32)
    # DC (f=0) correction: out[b, :] += (2/N) * (sum_j d)(sum_j b), a rank-1
    # term folded into the host unshard sum.
    kv = K.sum(axis=1)
    out += (2.0 / N) * ((des @ kv) * (body @ kv))[:, None]
    return out


# revision 36
# speedup vs baseline: 1.0758x; 1.0017x over previous
r"""Circulant layer kernel for Trainium2 (8 NeuronCores).

Math: reference computes mv1 + mv2 where
  mv1 = batch_circulant(b) @ d,  mv2 = batch_circulant(d) @ b,
with d = des @ K, b = body @ K.  Both are the circular convolution of d and b
(circular convolution is commutative), so  out = 2 * circconv(d, b).

circconv via DFT:  out = 2 * Re(IDFT(DFT(d) * DFT(b))).  d and b are REAL,
so the spectrum is conjugate-symmetric and only frequencies 0..512 are
needed; paired frequencies 1..511 carry weight 4/N in the inverse, the
self-conjugate f=512 carries 2/N, and the f=0 (DC) term is a rank-1
correction added on the host during the unshard sum.

Sharding: core c owns the 64 frequencies f in [64c+1, 64c+64] (core 7's
last is f=512, whose sin column is identically zero).  Per core:
  KC_c   = K @ CC_c            (1024k x 128s)   fused projection+fwd DFT
  DT_c   = KC_c^T @ des^T      (128s x 128b)    \  shares stationary weights
  BT_c   = KC_c^T @ body^T     (128s x 128b)    /
  PT_c   = complex-mult(DT_c, BT_c)             (64f x 2 x 128b)
  part_c = (PT_c^T @ G_c)                       (128b x 1024)  inverse DFT
Host sums the 8 partials and adds the DC term (unshard).

Key structural facts this implementation is built around:
- walrus allows ONE sync wait per instruction, so every consumer's
  dependencies must collapse onto a single producer engine.  DMA-fed
  operands consumed together with engine-produced data (dbt, g) are
  staged through Vector copies.
- a PSUM accumulation chain's first matmul clears has_written bits for
  its WHOLE 2KB bank, and the Tile scheduler reorders matmuls, so bank
  sharing is only safe inside ONE chain with explicit order edges.
  Stage 1 packs four kb regions per bank inside one ordered chain;
  stages 2/4 use fresh banks so no bank is ever re-read after a rewrite.
- stage 1 runs j-outer with all 8 accumulators live, so each arriving
  K chunk (4 chunked DMAs) is consumed immediately.
- PE warmup (HAM clock ramp) is folded into stage 1: zero matmuls
  accumulate into the kb0 region before the real contributions.
"""

import numpy as np

import concourse.bass as bass
import concourse.mybir as mybir
import concourse.tile as tile
from concourse.bass_utils import run_bass_kernel_spmd
from concourse.tile_rust import add_dep_helper

B = 128        # batch
D_IN = 1024    # input feature dim (contraction k)
N = 1024       # output feature dim (conv length j)
N_CORES = 8
SH = 64             # frequencies per core (complex, from the half spectrum)
S = 2 * SH          # freq slots per core: [0:SH]=real(cos), [SH:2SH]=imag(-sin)
JC = N // 128       # 8 j-chunks (contraction of stage 1)
KB = D_IN // 128    # 8 k-blocks (output partitions of stage 1)
QSPLIT = [2, 2, 2, 1, 1]   # j-chunks per ktcc DMA (small LAST chunks:
NQ = len(QSPLIT)           # minimal stage-1 tail after the stream ends)
QOFF = [sum(QSPLIT[:i]) for i in range(NQ)]

F32 = mybir.dt.float32
F32R = mybir.dt.float32r
BF16 = mybir.dt.bfloat16

# Matmul operand precision: "bf16" (fastest; ~5e-3 rel err), "f32r"
# (single-pass TF32-like; ~3e-4), "f32" (two-pass full fp32; ~7e-7).
import os as _os
MM_PREC = _os.environ.get("CIRC_MM_PREC", "bf16")
MM_DT = {"bf16": BF16, "f32r": F32R, "f32": F32}[MM_PREC]
N_WARM = int(_os.environ.get("CIRC_WARM", "10"))  # zero matmuls folded into stage 1


def _np_in(a):
    """Cast to the matmul precision; bf16 data is shipped packed in fp32
    words (DMA is element-rate-bound: 2-byte elements run at half rate)."""
    import ml_dtypes
    a = np.ascontiguousarray(np.asarray(a, dtype=np.float32))
    if MM_PREC != "bf16":
        return a
    bf = np.ascontiguousarray(a.astype(ml_dtypes.bfloat16))
    return bf.view(np.uint8).reshape(a.shape[0], -1).view(np.float32)

# Number of fp32 transport words per logical input element.
PACK = 2 if MM_PREC == "bf16" else 1
# Transport dtype: bf16 ships packed in fp32 words; f32/f32r ship natively.
TR_DT = F32 if MM_PREC == "bf16" else MM_DT
# Output transport: bf16 halves the store; f32/f32r debug modes store fp32.
OUT_DT = BF16 if MM_PREC == "bf16" else F32

XW = (D_IN + S) // PACK   # words per partition per j-chunk (kt | cc)
DW = 2 * B // PACK        # words per partition per k-block of (des^T|body^T)
GW = N // PACK            # words per partition per s-plane of G

# Stashed by kernel() for test harnesses that want profiling info.
LAST_RESULT = None

_nc_cache = {}


def _build_nc():
    """Build the (single-program) Bass module run on all 8 cores."""
    nc = bass.Bass(target_bir_lowering=True)

    # K^T and CC are packed together per j-chunk so each chunk DMA delivers
    # a self-sufficient unit of stage-1 work.  All inputs are host-packed
    # per SBUF partition: row p holds everything partition p receives.
    # chunk 0 carries dbt appended per partition (staging copies run early,
    # while Vector is otherwise idle); g rides its own small 64-partition
    # DMA placed mid-chain.
    EXTRA = [KB * DW if q == 0 else 0 for q in range(NQ)]
    ktcc_q = [nc.declare_dram_parameter(
                  f"ktcc{q}", [128, QSPLIT[q] * XW + EXTRA[q]], TR_DT, False)
              for q in range(NQ)]
    g_d = nc.declare_dram_parameter("g", [SH, 2 * GW], TR_DT, False)
    out_d = nc.declare_dram_parameter("out", [B, N], OUT_DT, isOutput=True)
    DEBUG = _os.environ.get("CIRC_DEBUG", "")
    kcdbg_d = (nc.declare_dram_parameter("kcdbg", [128, KB * S], F32, isOutput=True)
               if "kc" in DEBUG else None)
    dbdbg_d = (nc.declare_dram_parameter("dbdbg", [SH, 2 * 2 * B], F32, isOutput=True)
               if "db" in DEBUG else None)

    with tile.TileContext(nc) as tc:
        with (
            tc.tile_pool(name="main", bufs=1) as pool,
            tc.tile_pool(name="psum", bufs=1, space="PSUM") as pp,
        ):
            # ---- inputs -> SBUF ----
            # All input transfers ride ONE serial SP chain (a serial chain
            # pipelines; parallel channels all pay the full proxy latency).
            # dbt/g ride in the middle: late enough not to delay the first
            # K chunks, early enough that staging copies beat stage 2.
            dbg_dmas = []
            ktcc_sb = [pool.tile([128, QSPLIT[q] * XW + EXTRA[q]],
                                 TR_DT, tag=f"ktcc{q}", name=f"ktcc{q}")
                       for q in range(NQ)]
            g_raw = pool.tile([SH, 2 * GW], TR_DT, tag="gr", name="gr")
            in_dmas = [nc.sync.dma_start(ktcc_sb[q][:], ktcc_q[q][:, :])
                       for q in range(2)]
            in_dmas.append(nc.sync.dma_start(g_raw[:], g_d[:, :]))
            in_dmas.extend(nc.sync.dma_start(ktcc_sb[q][:], ktcc_q[q][:, :])
                           for q in range(2, NQ))

            # j-chunk views into the flat per-DMA tiles
            _jq = {}
            for q in range(NQ):
                kpart = ktcc_sb[q][:, :QSPLIT[q] * XW]
                v = kpart.bitcast(MM_DT).rearrange(
                    "p (c x) -> p c x", c=QSPLIT[q])
                for r in range(QSPLIT[q]):
                    _jq[QOFF[q] + r] = v[:, r, :]
            kt_sb = [_jq[j][:, :D_IN] for j in range(JC)]
            cc_sb = [_jq[j][:, D_IN:] for j in range(JC)]

            # Staging copies: DMA-sem -> Vector-sem so stage-2/4 matmuls
            # need only a single (Vector) wait.
            dbt_sb = pool.tile([128, KB, DW], TR_DT, tag="dbt", name="dbt")
            g_sb = pool.tile([SH, 2, GW], TR_DT, tag="g", name="g")
            nc.vector.tensor_copy(
                dbt_sb[:], ktcc_sb[0][:, QSPLIT[0] * XW:]
                .rearrange("p (kb w) -> p kb w", kb=KB))
            nc.vector.tensor_copy(
                g_sb[:], g_raw[:].rearrange("p (sb w) -> p sb w", sb=2))
            dbt_v = dbt_sb.bitcast(MM_DT)   # [128, KB, 2B]
            g_v = g_sb.bitcast(MM_DT)       # [SH, 2, N]

            # ---- PSUM: stage 1 in banks 0-1 (kb quads share a bank inside
            # one ordered chain), stage 2 banks 2-3, stage 4 banks 4-5.
            s1_ps = pp.tile([128, 2, 512], F32, tag="s1", name="s1")
            kc_ps = lambda kb: s1_ps[:, kb // 4, (kb % 4) * S:(kb % 4) * S + S]
            db_ps = pp.tile([SH, 2, 512], F32, tag="dbp", name="dbp")
            o_ps = [pp.tile([128, 512], F32, tag=f"op{h}", name=f"op{h}")
                    for h in range(2)]

            # ---- stage 1 (with folded warmup): KC[k,s] = sum_j KT[j,k]*CC[j,s]
            wz = pool.tile([128, 128 + S], BF16, tag="wz", name="wz")
            nc.gpsimd.memset(wz[:], 0.0)
            order = []  # explicit program-order edges (free: sync=False)
            for w in range(N_WARM):
                order.append(nc.tensor.matmul(
                    kc_ps(0), wz[:, :128], wz[:, 128:128 + S],
                    start=(w == 0), stop=False,
                    skip_group_check=True))
            for j in range(JC):
                for kb in range(KB):
                    # Bank clears: warmup mm 0 cleared bank 0; bank 1 is
                    # cleared by its first-ever matmul (j0, kb4).
                    order.append(nc.tensor.matmul(
                        kc_ps(kb),
                        kt_sb[j][:, kb * 128:(kb + 1) * 128],
                        cc_sb[j][:],
                        start=(j == 0 and (kb == 4 or (kb == 0 and N_WARM == 0))),
                        stop=(j == JC - 1),
                        skip_group_check=True,
                    ))
            for a, b_ in zip(order, order[1:]):
                add_dep_helper(b_.ins, a.ins, sync=False,
                               reason="stage1: keep bank-sharing chains ordered")

            # ---- drain stage-1 psum with ONE Vector copy ----
            # s1_ps viewed [128, 8, S] is exactly kb-major order.
            kc_sb = pool.tile([128, KB, S], MM_DT, tag="kc", name="kc")
            nc.vector.tensor_copy(
                kc_sb[:, 0:4, :], s1_ps[:, 0, :].rearrange("p (r s) -> p r s", r=4))
            nc.vector.tensor_copy(
                kc_sb[:, 4:8, :], s1_ps[:, 1, :].rearrange("p (r s) -> p r s", r=4))
            if kcdbg_d is not None:
                kcf = pool.tile([128, KB * S], F32, tag="kcf", name="kcf")
                nc.vector.tensor_copy(kcf[:].rearrange("p (kb s) -> p kb s", kb=KB), kc_sb[:])
                dbg_dmas.append(nc.scalar.dma_start(kcdbg_d[:, :], kcf[:]))

            # ---- stage 2: DT/BT = KC^T @ (des^T|body^T) ----
            # cos rows -> chain [SH, 2B] in bank 2; sin rows -> bank 3.
            for kb in range(KB):
                for hh in range(2):
                    nc.tensor.matmul(
                        db_ps[:, hh, :2 * B],
                        kc_sb[:, kb, hh * SH:(hh + 1) * SH],
                        dbt_v[:, kb, :],
                        start=(kb == 0),
                        stop=(kb == KB - 1),
                    )

            # ---- stage 3: complex pointwise multiply ----
            # t01 = [Dr*Br, Dr*Bi], t23 = [Di*Bi, Di*Br]
            # Pr = t01[0] - t23[0],  Pi = t01[1] + t23[1]
            dd = pool.tile([SH, 2, B], F32, tag="dd", name="dd")
            nc.vector.tensor_copy(dd[:], db_ps[:, :, :B])
            if dbdbg_d is not None:
                dball = pool.tile([SH, 2, 2 * B], F32, tag="dball", name="dball")
                nc.vector.tensor_copy(dball[:], db_ps[:, :, :2 * B])
                dbg_dmas.append(nc.gpsimd.dma_start(
                    dbdbg_d[:, :], dball[:].rearrange("p a b -> p (a b)")))
            t01 = pool.tile([SH, 2, B], F32, tag="t01", name="t01")
            t23 = pool.tile([SH, 2, B], F32, tag="t23", name="t23")
            pt = pool.tile([SH, 2, B], MM_DT, tag="pt", name="pt")
            dr_b = dd[:, 0, :][:, None, :].to_broadcast((SH, 2, B))
            di_b = dd[:, 1, :][:, None, :].to_broadcast((SH, 2, B))
            nc.vector.tensor_mul(t01[:], dr_b, db_ps[:, :, B:2 * B])
            nc.vector.tensor_mul(t23[:], di_b, db_ps[:, ::-1, B:2 * B])
            nc.vector.tensor_sub(pt[:, 0, :], t01[:, 0, :], t23[:, 0, :])
            nc.gpsimd.tensor_add(pt[:, 1, :], t01[:, 1, :], t23[:, 1, :])

            # ---- stage 4: part = PT^T @ G; store each half as it drains ----
            out_sb = pool.tile([128, N], OUT_DT, tag="outsb", name="outsb")
            last_mm = None
            for h in range(2):
                for sb in range(2):
                    last_mm = nc.tensor.matmul(
                        o_ps[h][:],
                        pt[:, sb, :],
                        g_v[:, sb, h * 512:(h + 1) * 512],
                        start=(sb == 0),
                        stop=(sb == 1),
                    )
            cp0 = nc.vector.tensor_copy(out_sb[:, :512], o_ps[0][:])
            store_a = nc.sync.dma_start(out_d[:, :512], out_sb[:, :512])
            cp1 = nc.scalar.copy(out_sb[:, 512:], o_ps[1][:])
            store_b = nc.scalar.dma_start(out_d[:, 512:], out_sb[:, 512:])

            # TileContext's exit emits one tail Drain waiting on every
            # outstanding semaphore; walrus caps instructions at ONE sync
            # wait.  Pre-absorb every tick into SP's clock with a chain of
            # single-wait drains so the tail drain needs none.
            prev = None
            for dep in [*in_dmas, *dbg_dmas, store_a, store_b, last_mm, cp0, cp1]:
                dr = nc.sync.drain(fusable=False)
                add_dep_helper(dr.ins, dep.ins, sync=True,
                               reason="tail: absorb tick into SP clock")
                if prev is not None:
                    add_dep_helper(dr.ins, prev.ins, sync=False,
                                   reason="tail: keep drain chain ordered")
                prev = dr

    return nc


def _dft_constants():
    """Per-core forward (CC) and inverse (G) half-spectrum DFT matrices.

    Core c owns f in [64c+1, 64c+64].  Inverse weights: 4/N for paired
    frequencies 1..511, 2/N for the self-conjugate f=512.  (f=0 is the
    host-side DC correction.)
    """
    j = np.arange(N, dtype=np.float64)
    ccs, gs = [], []
    for c in range(N_CORES):
        f = np.arange(SH * c + 1, SH * c + SH + 1, dtype=np.float64)
        ang = 2.0 * np.pi * np.outer(j, f) / N          # (j, f)
        cc = np.concatenate([np.cos(ang), -np.sin(ang)], axis=1)   # (N, S)
        w = np.full(SH, 4.0 / N)
        if c == N_CORES - 1:
            w[-1] = 2.0 / N                              # f = 512
        angT = ang.T                                     # (f, k)
        gr = w[:, None] * np.cos(angT)
        gi = -w[:, None] * np.sin(angT)
        gmat = np.concatenate([gr, gi], axis=1)          # (SH, 2N): [cos|sin]
        ccs.append(np.ascontiguousarray(cc, dtype=np.float32))
        gs.append(np.ascontiguousarray(gmat, dtype=np.float32))
    return ccs, gs


def _partition_pack(a):
    """(R, W) with R = n*128 -> (128, n, W): row p = stack of chunk rows p."""
    r, w = a.shape
    n = r // 128
    return np.ascontiguousarray(a.reshape(n, 128, w).transpose(1, 0, 2))


def kernel(des, body, kernel):
    global LAST_RESULT
    des = np.asarray(des, dtype=np.float32)
    body = np.asarray(body, dtype=np.float32)
    K = np.asarray(kernel, dtype=np.float32)
    kt_np = K.T  # (j, k)
    dbt_np = _partition_pack(_np_in(np.concatenate(
        [des.T, body.T], axis=1))).reshape(128, KB * DW)
    ccs, gs = _dft_constants()
    in_maps = []
    for c in range(N_CORES):
        ktcc = _partition_pack(_np_in(np.concatenate([kt_np, ccs[c]], axis=1)))
        m = {f"ktcc{q}": np.ascontiguousarray(
                ktcc[:, QOFF[q]:QOFF[q] + QSPLIT[q], :]).reshape(128, QSPLIT[q] * XW)
             for q in range(NQ)}
        m["ktcc0"] = np.ascontiguousarray(np.concatenate([m["ktcc0"], dbt_np], axis=1))
        m["g"] = np.ascontiguousarray(_np_in(gs[c]))
        in_maps.append(m)

    if "nc" not in _nc_cache:
        _nc_cache["nc"] = _build_nc()
    nc = _nc_cache["nc"]

    res = run_bass_kernel_spmd(nc, in_maps, list(range(N_CORES)))
    LAST_RESULT = res
    out = np.zeros((B, N), dtype=np.float32)
    for r in res.results:
        out += np.asarray(r["out"], dtype=np.float32)
    # DC (f=0) correction: out[b, :] += (2/N) * (sum_j d)(sum_j b), a rank-1
    # term folded into the host unshard sum.
    kv = K.sum(axis=1)
    out += (2.0 / N) * ((des @ kv) * (body @ kv))[:, None]
    return out


# revision 37
# speedup vs baseline: 1.1383x; 1.0581x over previous
r"""Circulant layer kernel for Trainium2 (8 NeuronCores).

Math: reference computes mv1 + mv2 where
  mv1 = batch_circulant(b) @ d,  mv2 = batch_circulant(d) @ b,
with d = des @ K, b = body @ K.  Both are the circular convolution of d and b
(circular convolution is commutative), so  out = 2 * circconv(d, b).

circconv via DFT:  out = 2 * Re(IDFT(DFT(d) * DFT(b))).  d and b are REAL,
so the spectrum is conjugate-symmetric and only frequencies 0..512 are
needed; paired frequencies 1..511 carry weight 4/N in the inverse, the
self-conjugate f=512 carries 2/N, and the f=0 (DC) term is a rank-1
correction added on the host during the unshard sum.

Sharding: core c owns the 64 frequencies f in [64c+1, 64c+64] (core 7's
last is f=512, whose sin column is identically zero).  Per core:
  KC_c   = K @ CC_c            (1024k x 128s)   fused projection+fwd DFT
  DT_c   = KC_c^T @ des^T      (128s x 128b)    \  shares stationary weights
  BT_c   = KC_c^T @ body^T     (128s x 128b)    /
  PT_c   = complex-mult(DT_c, BT_c)             (64f x 2 x 128b)
  part_c = (PT_c^T @ G_c)                       (128b x 1024)  inverse DFT
Host sums the 8 partials and adds the DC term (unshard).

Key structural facts this implementation is built around:
- walrus allows ONE sync wait per instruction, so every consumer's
  dependencies must collapse onto a single producer engine.  DMA-fed
  operands consumed together with engine-produced data (dbt, g) are
  staged through Vector copies.
- a PSUM accumulation chain's first matmul clears has_written bits for
  its WHOLE 2KB bank, and the Tile scheduler reorders matmuls, so bank
  sharing is only safe inside ONE chain with explicit order edges.
  Stage 1 packs four kb regions per bank inside one ordered chain;
  stages 2/4 use fresh banks so no bank is ever re-read after a rewrite.
- stage 1 runs j-outer with all 8 accumulators live, so each arriving
  K chunk (4 chunked DMAs) is consumed immediately.
- PE warmup (HAM clock ramp) is folded into stage 1: zero matmuls
  accumulate into the kb0 region before the real contributions.
"""

import numpy as np

import concourse.bass as bass
import concourse.mybir as mybir
import concourse.tile as tile
from concourse.bass_utils import run_bass_kernel_spmd
from concourse.tile_rust import add_dep_helper

B = 128        # batch
D_IN = 1024    # input feature dim (contraction k)
N = 1024       # output feature dim (conv length j)
N_CORES = 8
SH = 64             # frequencies per core (complex, from the half spectrum)
S = 2 * SH          # freq slots per core: [0:SH]=real(cos), [SH:2SH]=imag(-sin)
JC = N // 128       # 8 j-chunks (contraction of stage 1)
KB = D_IN // 128    # 8 k-blocks (output partitions of stage 1)
QSPLIT = [2, 2, 2, 1, 1]   # j-chunks per ktcc DMA (small LAST chunks:
NQ = len(QSPLIT)           # minimal stage-1 tail after the stream ends)
QOFF = [sum(QSPLIT[:i]) for i in range(NQ)]

F32 = mybir.dt.float32
F32R = mybir.dt.float32r
BF16 = mybir.dt.bfloat16

# Matmul operand precision: "bf16" (fastest; ~5e-3 rel err), "f32r"
# (single-pass TF32-like; ~3e-4), "f32" (two-pass full fp32; ~7e-7).
import os as _os
MM_PREC = _os.environ.get("CIRC_MM_PREC", "bf16")
MM_DT = {"bf16": BF16, "f32r": F32R, "f32": F32}[MM_PREC]
N_WARM = int(_os.environ.get("CIRC_WARM", "10"))  # zero matmuls folded into stage 1


def _np_in(a):
    """Cast to the matmul precision; bf16 data is shipped packed in fp32
    words (DMA is element-rate-bound: 2-byte elements run at half rate)."""
    import ml_dtypes
    a = np.ascontiguousarray(np.asarray(a, dtype=np.float32))
    if MM_PREC != "bf16":
        return a
    bf = np.ascontiguousarray(a.astype(ml_dtypes.bfloat16))
    return bf.view(np.uint8).reshape(a.shape[0], -1).view(np.float32)

# Number of fp32 transport words per logical input element.
PACK = 2 if MM_PREC == "bf16" else 1
# Transport dtype: bf16 ships packed in fp32 words; f32/f32r ship natively.
TR_DT = F32 if MM_PREC == "bf16" else MM_DT
# Output transport: bf16 halves the store; f32/f32r debug modes store fp32.
OUT_DT = BF16 if MM_PREC == "bf16" else F32

XW = (D_IN + S) // PACK   # words per partition per j-chunk (kt | cc)
DW = 2 * B // PACK        # words per partition per k-block of (des^T|body^T)
GW = N // PACK            # words per partition per s-plane of G

# Stashed by kernel() for test harnesses that want profiling info.
LAST_RESULT = None

_nc_cache = {}


def _build_nc():
    """Build the (single-program) Bass module run on all 8 cores."""
    nc = bass.Bass(target_bir_lowering=True)

    # K^T and CC are packed together per j-chunk so each chunk DMA delivers
    # a self-sufficient unit of stage-1 work.  All inputs are host-packed
    # per SBUF partition: row p holds everything partition p receives.
    # chunk 0 carries dbt appended per partition (staging copies run early,
    # while Vector is otherwise idle); g rides its own small 64-partition
    # DMA placed mid-chain.
    EXTRA = [KB * DW if q == 0 else 0 for q in range(NQ)]
    ktcc_q = [nc.declare_dram_parameter(
                  f"ktcc{q}", [128, QSPLIT[q] * XW + EXTRA[q]], TR_DT, False)
              for q in range(NQ)]
    g_d = nc.declare_dram_parameter("g", [SH, 2 * GW], TR_DT, False)
    out_d = nc.declare_dram_parameter("out", [B, N], OUT_DT, isOutput=True)
    DEBUG = _os.environ.get("CIRC_DEBUG", "")
    kcdbg_d = (nc.declare_dram_parameter("kcdbg", [128, KB * S], F32, isOutput=True)
               if "kc" in DEBUG else None)
    dbdbg_d = (nc.declare_dram_parameter("dbdbg", [SH, 2 * 2 * B], F32, isOutput=True)
               if "db" in DEBUG else None)

    with tile.TileContext(nc) as tc:
        with (
            tc.tile_pool(name="main", bufs=1) as pool,
            tc.tile_pool(name="psum", bufs=1, space="PSUM") as pp,
        ):
            # ---- inputs -> SBUF ----
            # All input transfers ride ONE serial SP chain (a serial chain
            # pipelines; parallel channels all pay the full proxy latency).
            # dbt/g ride in the middle: late enough not to delay the first
            # K chunks, early enough that staging copies beat stage 2.
            dbg_dmas = []
            ktcc_sb = [pool.tile([128, QSPLIT[q] * XW + EXTRA[q]],
                                 TR_DT, tag=f"ktcc{q}", name=f"ktcc{q}")
                       for q in range(NQ)]
            g_raw = pool.tile([SH, 2 * GW], TR_DT, tag="gr", name="gr")
            in_dmas = [nc.sync.dma_start(ktcc_sb[q][:], ktcc_q[q][:, :])
                       for q in range(2)]
            in_dmas.append(nc.sync.dma_start(g_raw[:], g_d[:, :]))
            in_dmas.extend(nc.sync.dma_start(ktcc_sb[q][:], ktcc_q[q][:, :])
                           for q in range(2, NQ))

            # j-chunk views into the flat per-DMA tiles
            _jq = {}
            for q in range(NQ):
                kpart = ktcc_sb[q][:, :QSPLIT[q] * XW]
                v = kpart.bitcast(MM_DT).rearrange(
                    "p (c x) -> p c x", c=QSPLIT[q])
                for r in range(QSPLIT[q]):
                    _jq[QOFF[q] + r] = v[:, r, :]
            kt_sb = [_jq[j][:, :D_IN] for j in range(JC)]
            cc_sb = [_jq[j][:, D_IN:] for j in range(JC)]

            # Staging copies: DMA-sem -> Vector-sem so stage-2/4 matmuls
            # need only a single (Vector) wait.
            dbt_sb = pool.tile([128, KB, DW], TR_DT, tag="dbt", name="dbt")
            g_sb = pool.tile([SH, 2, GW], TR_DT, tag="g", name="g")
            nc.vector.tensor_copy(
                dbt_sb[:], ktcc_sb[0][:, QSPLIT[0] * XW:]
                .rearrange("p (kb w) -> p kb w", kb=KB))
            nc.vector.tensor_copy(
                g_sb[:], g_raw[:].rearrange("p (sb w) -> p sb w", sb=2))
            dbt_v = dbt_sb.bitcast(MM_DT)   # [128, KB, 2B]
            g_v = g_sb.bitcast(MM_DT)       # [SH, 2, N]

            # ---- PSUM: stage 1 in banks 0-1 (kb quads share a bank inside
            # one ordered chain), stage 2 banks 2-3, stage 4 banks 4-5.
            s1_ps = pp.tile([128, 2, 512], F32, tag="s1", name="s1")
            kc_ps = lambda kb: s1_ps[:, kb // 4, (kb % 4) * S:(kb % 4) * S + S]
            db_ps = pp.tile([SH, 2, 512], F32, tag="dbp", name="dbp")
            o_ps = [pp.tile([128, 512], F32, tag=f"op{h}", name=f"op{h}")
                    for h in range(2)]

            # ---- stage 1 (with folded warmup): KC[k,s] = sum_j KT[j,k]*CC[j,s]
            wz = pool.tile([128, 128 + S], BF16, tag="wz", name="wz")
            nc.gpsimd.memset(wz[:], 0.0)
            order = []  # explicit program-order edges (free: sync=False)
            for w in range(N_WARM):
                order.append(nc.tensor.matmul(
                    kc_ps(0), wz[:, :128], wz[:, 128:128 + S],
                    start=(w == 0), stop=False,
                    skip_group_check=True))
            for j in range(JC):
                for kb in range(KB):
                    # Bank clears: warmup mm 0 cleared bank 0; bank 1 is
                    # cleared by its first-ever matmul (j0, kb4).
                    order.append(nc.tensor.matmul(
                        kc_ps(kb),
                        kt_sb[j][:, kb * 128:(kb + 1) * 128],
                        cc_sb[j][:],
                        start=(j == 0 and (kb == 4 or (kb == 0 and N_WARM == 0))),
                        stop=(j == JC - 1),
                        skip_group_check=True,
                    ))
            for a, b_ in zip(order, order[1:]):
                add_dep_helper(b_.ins, a.ins, sync=False,
                               reason="stage1: keep bank-sharing chains ordered")

            # ---- drain stage-1 psum with ONE Vector copy ----
            # s1_ps viewed [128, 8, S] is exactly kb-major order.
            kc_sb = pool.tile([128, KB, S], MM_DT, tag="kc", name="kc")
            for hc in range(4):
                nc.vector.tensor_copy(
                    kc_sb[:, 2 * hc:2 * hc + 2, :],
                    s1_ps[:, hc // 2, (hc % 2) * 2 * S:((hc % 2) * 2 + 2) * S]
                    .rearrange("p (r s) -> p r s", r=2))
            if kcdbg_d is not None:
                kcf = pool.tile([128, KB * S], F32, tag="kcf", name="kcf")
                nc.vector.tensor_copy(kcf[:].rearrange("p (kb s) -> p kb s", kb=KB), kc_sb[:])
                dbg_dmas.append(nc.scalar.dma_start(kcdbg_d[:, :], kcf[:]))

            # ---- stage 2: DT/BT = KC^T @ (des^T|body^T) ----
            # cos rows -> chain [SH, 2B] in bank 2; sin rows -> bank 3.
            for kb in range(KB):
                for hh in range(2):
                    nc.tensor.matmul(
                        db_ps[:, hh, :2 * B],
                        kc_sb[:, kb, hh * SH:(hh + 1) * SH],
                        dbt_v[:, kb, :],
                        start=(kb == 0),
                        stop=(kb == KB - 1),
                    )

            # ---- stage 3: complex pointwise multiply ----
            # t01 = [Dr*Br, Dr*Bi], t23 = [Di*Bi, Di*Br]
            # Pr = t01[0] - t23[0],  Pi = t01[1] + t23[1]
            dd = pool.tile([SH, 2, B], F32, tag="dd", name="dd")
            nc.vector.tensor_copy(dd[:], db_ps[:, :, :B])
            if dbdbg_d is not None:
                dball = pool.tile([SH, 2, 2 * B], F32, tag="dball", name="dball")
                nc.vector.tensor_copy(dball[:], db_ps[:, :, :2 * B])
                dbg_dmas.append(nc.gpsimd.dma_start(
                    dbdbg_d[:, :], dball[:].rearrange("p a b -> p (a b)")))
            t01 = pool.tile([SH, 2, B], F32, tag="t01", name="t01")
            t23 = pool.tile([SH, 2, B], F32, tag="t23", name="t23")
            pt = pool.tile([SH, 2, B], MM_DT, tag="pt", name="pt")
            dr_b = dd[:, 0, :][:, None, :].to_broadcast((SH, 2, B))
            di_b = dd[:, 1, :][:, None, :].to_broadcast((SH, 2, B))
            nc.vector.tensor_mul(t01[:], dr_b, db_ps[:, :, B:2 * B])
            nc.vector.tensor_mul(t23[:], di_b, db_ps[:, ::-1, B:2 * B])
            nc.vector.tensor_sub(pt[:, 0, :], t01[:, 0, :], t23[:, 0, :])
            nc.gpsimd.tensor_add(pt[:, 1, :], t01[:, 1, :], t23[:, 1, :])

            # ---- stage 4: part = PT^T @ G; store each half as it drains ----
            out_sb = pool.tile([128, N], OUT_DT, tag="outsb", name="outsb")
            last_mm = None
            for h in range(2):
                for sb in range(2):
                    last_mm = nc.tensor.matmul(
                        o_ps[h][:],
                        pt[:, sb, :],
                        g_v[:, sb, h * 512:(h + 1) * 512],
                        start=(sb == 0),
                        stop=(sb == 1),
                    )
            cp0 = nc.vector.tensor_copy(out_sb[:, :512], o_ps[0][:])
            store_a = nc.sync.dma_start(out_d[:, :512], out_sb[:, :512])
            cp1 = nc.scalar.copy(out_sb[:, 512:], o_ps[1][:])
            store_b = nc.scalar.dma_start(out_d[:, 512:], out_sb[:, 512:])

            # TileContext's exit emits one tail Drain waiting on every
            # outstanding semaphore; walrus caps instructions at ONE sync
            # wait.  Pre-absorb every tick into SP's clock with a chain of
            # single-wait drains so the tail drain needs none.
            prev = None
            for dep in [*in_dmas, *dbg_dmas, store_a, store_b, last_mm, cp0, cp1]:
                dr = nc.sync.drain(fusable=False)
                add_dep_helper(dr.ins, dep.ins, sync=True,
                               reason="tail: absorb tick into SP clock")
                if prev is not None:
                    add_dep_helper(dr.ins, prev.ins, sync=False,
                                   reason="tail: keep drain chain ordered")
                prev = dr

    return nc


def _dft_constants():
    """Per-core forward (CC) and inverse (G) half-spectrum DFT matrices.

    Core c owns f in [64c+1, 64c+64].  Inverse weights: 4/N for paired
    frequencies 1..511, 2/N for the self-conjugate f=512.  (f=0 is the
    host-side DC correction.)
    """
    j = np.arange(N, dtype=np.float64)
    ccs, gs = [], []
    for c in range(N_CORES):
        f = np.arange(SH * c + 1, SH * c + SH + 1, dtype=np.float64)
        ang = 2.0 * np.pi * np.outer(j, f) / N          # (j, f)
        cc = np.concatenate([np.cos(ang), -np.sin(ang)], axis=1)   # (N, S)
        w = np.full(SH, 4.0 / N)
        if c == N_CORES - 1:
            w[-1] = 2.0 / N                              # f = 512
        angT = ang.T                                     # (f, k)
        gr = w[:, None] * np.cos(angT)
        gi = -w[:, None] * np.sin(angT)
        gmat = np.concatenate([gr, gi], axis=1)          # (SH, 2N): [cos|sin]
        ccs.append(np.ascontiguousarray(cc, dtype=np.float32))
        gs.append(np.ascontiguousarray(gmat, dtype=np.float32))
    return ccs, gs


def _partition_pack(a):
    """(R, W) with R = n*128 -> (128, n, W): row p = stack of chunk rows p."""
    r, w = a.shape
    n = r // 128
    return np.ascontiguousarray(a.reshape(n, 128, w).transpose(1, 0, 2))


def kernel(des, body, kernel):
    global LAST_RESULT
    des = np.asarray(des, dtype=np.float32)
    body = np.asarray(body, dtype=np.float32)
    K = np.asarray(kernel, dtype=np.float32)
    kt_np = K.T  # (j, k)
    dbt_np = _partition_pack(_np_in(np.concatenate(
        [des.T, body.T], axis=1))).reshape(128, KB * DW)
    ccs, gs = _dft_constants()
    in_maps = []
    for c in range(N_CORES):
        ktcc = _partition_pack(_np_in(np.concatenate([kt_np, ccs[c]], axis=1)))
        m = {f"ktcc{q}": np.ascontiguousarray(
                ktcc[:, QOFF[q]:QOFF[q] + QSPLIT[q], :]).reshape(128, QSPLIT[q] * XW)
             for q in range(NQ)}
        m["ktcc0"] = np.ascontiguousarray(np.concatenate([m["ktcc0"], dbt_np], axis=1))
        m["g"] = np.ascontiguousarray(_np_in(gs[c]))
        in_maps.append(m)

    if "nc" not in _nc_cache:
        _nc_cache["nc"] = _build_nc()
    nc = _nc_cache["nc"]

    res = run_bass_kernel_spmd(nc, in_maps, list(range(N_CORES)))
    LAST_RESULT = res
    out = np.zeros((B, N), dtype=np.float32)
    for r in res.results:
        out += np.asarray(r["out"], dtype=np.float32)
    # DC (f=0) correction: out[b, :] += (2/N) * (sum_j d)(sum_j b), a rank-1
    # term folded into the host unshard sum.
    kv = K.sum(axis=1)
    out += (2.0 / N) * ((des @ kv) * (body @ kv))[:, None]
    return out
